# revision 1
# baseline (speedup 1.0000x reference)
"""Bass/Trainium2 kernel for a 6-layer GPT-style transformer (BigramLanguageModel).

Contract: kernel(**inputs) takes the FULL unsharded inputs from
reference.setup_inputs() and returns the FULL [32, 512, 65] fp32 logits.

Sharding: data-parallel over batch. Each of the 8 NeuronCores runs the whole
model on 4 of the 32 sequences (params replicated); outputs are concatenated
on the host. No collectives.

Device-side design (per core, 2048 tokens):
 - residual stream x: token-major SBUF [128, 16, 384] (tile t = seq*4 + block)
 - LayerNorm: bn_stats/bn_aggr (DVE) + fused (x-m)*rstd tensor_scalar; the
   ln gains/biases are folded into the following matmul weights on the host
   (exact: (x_hat*g + b) @ W == x_hat @ (g[:,None]*W) + b@W).
 - h is transposed to E-major hT [128, 3, 2048] with PE transposes (batched
   4 per PSUM bank, single copy-out) since the tensor engine contracts over
   the partition dim.
 - QKV: QT/KT per head-pair [128, 2048] = Wpair.T @ hT; V token-major.
 - attention per (head, seq): S^T blocks [128 k, width] with 128-block causal
   skipping; exp(scale*S) on ACT directly from PSUM; binary lower-tri mask on
   the diagonal block; attention-out computed transposed (OT [64, 512]) with a
   stationary operand [ones64 | V_h] so 64 PSUM rows replicate the softmax
   denominator -> DVE reciprocal + one tensor_tensor multiply writes the
   normalized OT into the E-major concat buffer feeding the proj matmul.
 - MLP: mlpT = W1.T @ h2T (E-major), relu fused into the PSUM->SBUF copy,
   W2 with mlpT chunks as stationary operand, token-major out + residual add.
 - logits: final LN -> xfT -> x @ Wlm per token tile, DMA out [2048, 65].
"""

import sys

for _p in ("/opt/trn_rl_repo", "/opt/pypackages"):
    if _p not in sys.path:
        sys.path.insert(0, _p)

import numpy as np

import concourse.bass as bass
import concourse.tile as tile
from concourse import bacc, mybir
from concourse.bass_utils import run_bass_kernel_spmd

F32 = mybir.dt.float32
F32R = mybir.dt.float32r
BF16 = mybir.dt.bfloat16

N_EMBED = 384
CONTEXT = 512
N_HEADS = 6
HEAD_DIM = 64
N_LAYERS = 6
VOCAB = 65
B, T = 32, 512
LN_EPS = 1e-5
N_CORES = 8
B_LOC = B // N_CORES          # 4 sequences per core
N_TOK = B_LOC * T             # 2048 tokens per core
N_TILES = N_TOK // 128        # 16 token tiles
N_CHUNKS = N_EMBED // 128     # 3 E-chunks
N_MLP = 4 * N_EMBED           # 1536
N_MCHUNK = N_MLP // 128       # 12
SCALE = float(N_EMBED) ** -0.5
USE_FP32R = True    # single-pass PE matmuls (vs 2-pass LOW_HIGH fp32)
USE_FAST_RECIP = True
MDT = F32R if USE_FP32R else F32
DEBUG_L0 = False


MMDT = mybir.dt.float32r  # set below based on USE_FP32R


def _mm(ap):
    return ap

# V_aug free-layout: [V0 | ones | V1 | V2 | ones | V3 | V4 | ones | V5], 576 cols.
# Head h reads the contiguous 128-col window at V_OFF[h]; for even h the OT rows
# come first in the matmul output (rows 0:64) and the replicated denominator
# rows second; for odd h it is swapped.
V_POS = [0, 128, 192, 320, 384, 512]       # V_h column block
V_OFF = [0, 64, 192, 256, 384, 448]        # lhsT window start
V_OT_FIRST = [True, False, True, False, True, False]
V_W = 576


def _prep(inputs):
    """Host-side layout prep + exact LN folds. Returns (shared, per_core_idx)."""
    f = lambda a: np.ascontiguousarray(np.asarray(a), dtype=np.float32)
    idx = np.asarray(inputs["idx"])
    tok_emb, pos_emb = f(inputs["tok_emb"]), f(inputs["pos_emb"])
    Wq, Wk, Wv = f(inputs["Wq"]), f(inputs["Wk"]), f(inputs["Wv"])
    Wproj, bproj = f(inputs["Wproj"]), f(inputs["bproj"])
    W1, b1, W2, b2 = f(inputs["W1"]), f(inputs["b1"]), f(inputs["W2"]), f(inputs["b2"])
    ln1_g, ln1_b = f(inputs["ln1_g"]), f(inputs["ln1_b"])
    ln2_g, ln2_b = f(inputs["ln2_g"]), f(inputs["ln2_b"])
    lnf_g, lnf_b = f(inputs["lnf_g"]), f(inputs["lnf_b"])
    Wlm, blm = f(inputs["Wlm"]), f(inputs["blm"])

    L, H, E, D = N_LAYERS, N_HEADS, N_EMBED, HEAD_DIM

    # fold ln gains into the consuming weights (exact when g==1)
    Wq_f = ln1_g[:, None, :, None] * Wq          # [L,H,E,D]
    Wk_f = ln1_g[:, None, :, None] * Wk
    Wv_f = ln1_g[:, None, :, None] * Wv
    W1_f = ln2_g[:, :, None] * W1                # [L,E,4E]
    Wlm_f = lnf_g[:, None] * Wlm                 # [E,V]

    # ln biases propagate through the matmuls as constant bias vectors
    qb = np.einsum("le,lhed->lhd", ln1_b, Wq)    # [L,H,D]
    kb = np.einsum("le,lhed->lhd", ln1_b, Wk)
    vb = np.einsum("le,lhed->lhd", ln1_b, Wv)
    b1_eff = b1 + np.einsum("le,lem->lm", ln2_b, W1)    # [L,4E]
    blm_eff = blm + lnf_b @ Wlm                          # [V]

    # head-pair packed QT/KT weights: [L, 3, E, 128]
    wqp = np.concatenate([Wq_f[:, 0::2], Wq_f[:, 1::2]], axis=-1)  # [L,3,E,128]
    wkp = np.concatenate([Wk_f[:, 0::2], Wk_f[:, 1::2]], axis=-1)
    qbp = np.concatenate([qb[:, 0::2], qb[:, 1::2]], axis=-1)      # [L,3,128]
    kbp = np.concatenate([kb[:, 0::2], kb[:, 1::2]], axis=-1)
    wv_all = Wv_f.transpose(0, 2, 1, 3).reshape(L, E, H * D)       # [L,E,384]
    vb_all = vb.reshape(L, H * D)

    shared = dict(
        tok_emb=tok_emb,
        pos_emb=pos_emb,
        wqp=np.ascontiguousarray(wqp),
        wkp=np.ascontiguousarray(wkp),
        wv=np.ascontiguousarray(wv_all),
        wp=Wproj,
        w1=np.ascontiguousarray(W1_f),
        w2=W2,
        wlm=np.ascontiguousarray(Wlm_f),
        ident=np.eye(128, dtype=np.float32),
        iota=np.arange(VOCAB, dtype=np.float32).reshape(VOCAB, 1),
        # tri[k, j] = 1 where key k <= query j (within the diagonal block)
        tri=np.triu(np.ones((128, 128), dtype=np.float32)),
    )
    flags = dict(
        qb=qbp if np.any(qbp) else None,
        kb=kbp if np.any(kbp) else None,
        vb=np.broadcast_to(vb_all[:, None, :], (L, 128, H * D)).copy()
        if np.any(vb) else None,
        bp=np.broadcast_to(bproj[:, None, :], (L, 128, E)).copy()
        if np.any(bproj) else None,
        b1=np.ascontiguousarray(b1_eff.reshape(L, N_MCHUNK, 128).transpose(0, 2, 1))
        if np.any(b1_eff) else None,                    # [L,128,12] partition-major
        b2=np.broadcast_to(b2[:, None, :], (L, 128, E)).copy() if np.any(b2) else None,
        blm=np.broadcast_to(blm_eff[None, :], (128, VOCAB)).copy()
        if np.any(blm_eff) else None,
    )
    for k, v in flags.items():
        if v is not None:
            shared[k] = np.ascontiguousarray(v, dtype=np.float32)
    has = {k: (v is not None) for k, v in flags.items()}

    idx_f = idx.astype(np.float32).reshape(N_CORES, N_TOK)
    return shared, has, idx_f


def _build(has):
    nc = bacc.Bacc(trn_type="TRN2", debug=False, num_devices=N_CORES)
    d = {}
    d["idxf"] = nc.dram_tensor("idxf", [N_TOK], F32, kind="ExternalInput")
    d["tok_emb"] = nc.dram_tensor("tok_emb", [VOCAB, N_EMBED], F32, kind="ExternalInput")
    d["pos_emb"] = nc.dram_tensor("pos_emb", [CONTEXT, N_EMBED], F32, kind="ExternalInput")
    d["wqp"] = nc.dram_tensor("wqp", [N_LAYERS, 3, N_EMBED, 128], MDT, kind="ExternalInput")
    d["wkp"] = nc.dram_tensor("wkp", [N_LAYERS, 3, N_EMBED, 128], MDT, kind="ExternalInput")
    d["wv"] = nc.dram_tensor("wv", [N_LAYERS, N_EMBED, N_EMBED], MDT, kind="ExternalInput")
    d["wp"] = nc.dram_tensor("wp", [N_LAYERS, N_EMBED, N_EMBED], MDT, kind="ExternalInput")
    d["w1"] = nc.dram_tensor("w1", [N_LAYERS, N_EMBED, N_MLP], MDT, kind="ExternalInput")
    d["w2"] = nc.dram_tensor("w2", [N_LAYERS, N_MLP, N_EMBED], MDT, kind="ExternalInput")
    d["wlm"] = nc.dram_tensor("wlm", [N_EMBED, VOCAB], F32, kind="ExternalInput")
    d["ident"] = nc.dram_tensor("ident", [128, 128], F32, kind="ExternalInput")
    d["iota"] = nc.dram_tensor("iota", [VOCAB, 1], F32, kind="ExternalInput")
    d["tri"] = nc.dram_tensor("tri", [128, 128], F32, kind="ExternalInput")
    if has["qb"]:
        d["qb"] = nc.dram_tensor("qb", [N_LAYERS, 3, 128], F32, kind="ExternalInput")
    if has["kb"]:
        d["kb"] = nc.dram_tensor("kb", [N_LAYERS, 3, 128], F32, kind="ExternalInput")
    if has["vb"]:
        d["vb"] = nc.dram_tensor("vb", [N_LAYERS, 128, N_EMBED], F32, kind="ExternalInput")
    if has["bp"]:
        d["bp"] = nc.dram_tensor("bp", [N_LAYERS, 128, N_EMBED], F32, kind="ExternalInput")
    if has["b1"]:
        d["b1"] = nc.dram_tensor("b1", [N_LAYERS, 128, N_MCHUNK], F32, kind="ExternalInput")
    if has["b2"]:
        d["b2"] = nc.dram_tensor("b2", [N_LAYERS, 128, N_EMBED], F32, kind="ExternalInput")
    if has["blm"]:
        d["blm"] = nc.dram_tensor("blm", [128, VOCAB], F32, kind="ExternalInput")
    logits_d = nc.dram_tensor("logits", [N_TOK, VOCAB], F32, kind="ExternalOutput")
    dbg = {}
    if DEBUG_L0:
        dbg["x0"] = nc.dram_tensor("dbg_x0", [128, N_TILES * N_EMBED], F32, kind="ExternalOutput")
        dbg["ht"] = nc.dram_tensor("dbg_ht", [128, N_CHUNKS * N_TOK], MDT, kind="ExternalOutput")
        dbg["qt"] = nc.dram_tensor("dbg_qt", [128, N_TOK], F32, kind="ExternalOutput")
        dbg["kt"] = nc.dram_tensor("dbg_kt", [128, N_TOK], F32, kind="ExternalOutput")
        dbg["va"] = nc.dram_tensor("dbg_va", [128, N_TILES * V_W], MDT, kind="ExternalOutput")
        dbg["otc"] = nc.dram_tensor("dbg_otc", [128, N_CHUNKS * N_TOK], MDT, kind="ExternalOutput")
        dbg["x1"] = nc.dram_tensor("dbg_x1", [128, N_TILES * N_EMBED], F32, kind="ExternalOutput")
        dbg["x2"] = nc.dram_tensor("dbg_x2", [128, N_TILES * N_EMBED], F32, kind="ExternalOutput")
        dbg["at0"] = nc.dram_tensor("dbg_at0", [128, 512], MDT, kind="ExternalOutput")
        dbg["po0"] = nc.dram_tensor("dbg_po0", [128, 512], F32, kind="ExternalOutput")
        dbg["rho0"] = nc.dram_tensor("dbg_rho0", [64, 512], F32, kind="ExternalOutput")

    AF = mybir.ActivationFunctionType
    OP = mybir.AluOpType

    with tile.TileContext(nc) as tc:
        with tc.tile_pool(name="const", bufs=1) as cst, \
             tc.tile_pool(name="persist", bufs=1) as per, \
             tc.tile_pool(name="work", bufs=3) as wk, \
             tc.tile_pool(name="htile", bufs=4) as hp, \
             tc.tile_pool(name="wts", bufs=4) as wts, \
             tc.tile_pool(name="psA", bufs=4, space="PSUM") as psA, \
             tc.tile_pool(name="psB", bufs=2, space="PSUM") as psB, \
             tc.tile_pool(name="psC", bufs=2, space="PSUM") as psC:

            # ---- constants ----
            ident = cst.tile([128, 128], F32)
            nc.sync.dma_start(ident, d["ident"][:, :])
            iota = cst.tile([VOCAB, 1], F32)
            nc.sync.dma_start(iota, d["iota"][:, :])
            tri = cst.tile([128, 128], F32)
            nc.sync.dma_start(tri, d["tri"][:, :])
            eps_sb = cst.tile([128, 1], F32)
            nc.vector.memset(eps_sb, LN_EPS)
            tok_sb = cst.tile([VOCAB, N_EMBED], F32)
            nc.sync.dma_start(tok_sb, d["tok_emb"][:, :])

            bias_sb = {}
            for nm, shp in (("vb", [128, N_EMBED]), ("bp", [128, N_EMBED]),
                            ("b2", [128, N_EMBED])):
                if has[nm]:
                    bias_sb[nm] = cst.tile([128, N_LAYERS, shp[1]], F32)
                    nc.sync.dma_start(
                        bias_sb[nm],
                        d[nm].rearrange("l p e -> p l e"))
            if has["b1"]:
                bias_sb["b1"] = cst.tile([128, N_LAYERS, N_MCHUNK], F32)
                nc.sync.dma_start(bias_sb["b1"], d["b1"].rearrange("l p m -> p l m"))
            for nm in ("qb", "kb"):
                if has[nm]:
                    bias_sb[nm] = cst.tile([128, N_LAYERS, 3], F32)
                    nc.sync.dma_start(bias_sb[nm], d[nm].rearrange("l r p -> p l r"))
            if has["blm"]:
                bias_sb["blm"] = cst.tile([128, VOCAB], F32)
                nc.sync.dma_start(bias_sb["blm"], d["blm"][:, :])

            # ---- persistent activations ----
            x = per.tile([128, N_TILES, N_EMBED], F32)          # residual, token-major
            v_aug = per.tile([128, N_TILES, V_W], MDT)          # [V|ones] layout
            ones_blk = cst.tile([128, 64], F32)
            nc.vector.memset(ones_blk, 1.0)
            for c0 in (64, 256, 448):                           # ones stripes
                nc.vector.tensor_copy(
                    v_aug[:, :, c0:c0 + 64],
                    ones_blk[:, None, :].to_broadcast([128, N_TILES, 64]))

            # ---- embedding: x = onehot(idx) @ tok_emb + pos ----
            for t in range(N_TILES):
                idx_b = wk.tile([VOCAB, 128], F32, tag="idxb")
                nc.sync.dma_start(
                    idx_b,
                    bass.AP(tensor=d["idxf"], offset=t * 128,
                            ap=[[0, VOCAB], [1, 128]]))
                oh = wk.tile([VOCAB, 128], F32, tag="oh")
                nc.vector.tensor_scalar(out=oh, in0=idx_b, scalar1=iota,
                                        scalar2=None, op0=OP.is_equal)
                pe = psA.tile([128, 512], F32, tag="gen")
                nc.tensor.matmul(pe[:, :N_EMBED], lhsT=_mm(oh), rhs=_mm(tok_sb),
                                 start=True, stop=True)
                nc.scalar.copy(x[:, t, :], pe[:, :N_EMBED])
                nc.gpsimd.dma_start(
                    out=x[:, t, :],
                    in_=d["pos_emb"][(t % 4) * 128:(t % 4) * 128 + 128, :],
                    accum_op=OP.add)

            def layernorm_to(dst_hT, src_name):
                """bn_stats LN over x; writes E-major dst_hT [128, 3, 2048].

                Processed in groups of 4 token tiles (stats -> rstd -> apply ->
                transposes) so the PE gets transpose work early instead of
                idling through the whole DVE stats pass (keeps HAM warm)."""
                for tg in range(N_TILES // 4):
                    mv4 = wk.tile([128, 4, 2], F32, tag="mv" + src_name)
                    for dt in range(4):
                        st = wk.tile([128, 6], F32, tag="bnst")
                        nc.vector.bn_stats(out=st, in_=x[:, tg * 4 + dt, :])
                        nc.vector.bn_aggr(out=mv4[:, dt, :], in_=st)
                    sstd = wk.tile([128, 4], F32, tag="sstd")
                    nc.scalar.activation(out=sstd, in_=mv4[:, :, 1],
                                         func=AF.Sqrt, bias=eps_sb, scale=1.0)
                    rstd = wk.tile([128, 4], F32, tag="rstd")
                    nc.vector.reciprocal(out=rstd, in_=sstd)
                    hts = []
                    for dt in range(4):
                        t = tg * 4 + dt
                        ht = hp.tile([128, N_EMBED], F32, tag="h")
                        nc.vector.tensor_scalar(
                            out=ht, in0=x[:, t, :],
                            scalar1=mv4[:, dt, 0:1], scalar2=rstd[:, dt:dt + 1],
                            op0=OP.subtract, op1=OP.mult)
                        hts.append(ht)
                    for c in range(N_CHUNKS):
                        pt = psA.tile([128, 512], F32, tag="gen")
                        for dt in range(4):
                            nc.tensor.transpose(
                                pt[:, dt * 128:(dt + 1) * 128],
                                hts[dt][:, c * 128:(c + 1) * 128], ident)
                        nc.scalar.copy(dst_hT[:, c, tg * 512:(tg + 1) * 512], pt)

            if DEBUG_L0:
                nc.sync.dma_start(dbg["x0"][:, :], x.rearrange("p a b -> p (a b)"))
            for layer in range(N_LAYERS):
                hT = per.tile([128, N_CHUNKS, N_TOK], MDT, tag="ht")
                layernorm_to(hT, "ln1")
                if DEBUG_L0 and layer == 0:
                    nc.sync.dma_start(dbg["ht"][:, :], hT.rearrange("p a b -> p (a b)"))

                # ---- V (token-major, into v_aug) ----
                wv_c = []
                for c in range(N_CHUNKS):
                    w = wts.tile([128, N_EMBED], MDT, tag="wchk", bufs=3)
                    nc.sync.dma_start(w, d["wv"][layer, c * 128:(c + 1) * 128, :])
                    wv_c.append(w)
                for t in range(N_TILES):
                    pv = psA.tile([128, 512], F32, tag="gen")
                    for c in range(N_CHUNKS):
                        nc.tensor.matmul(pv[:, :N_EMBED],
                                         lhsT=_mm(hT[:, c, t * 128:(t + 1) * 128]),
                                         rhs=_mm(wv_c[c]),
                                         start=(c == 0), stop=(c == N_CHUNKS - 1))
                    src = pv[:, :N_EMBED].rearrange("p (g h j) -> p g h j", g=3, h=2)
                    dst = v_aug[:, t, :].rearrange(
                        "p (g h j) -> p g h j", g=3, h=3)[:, :, 0:3:2, :]
                    if has["vb"]:
                        nc.vector.tensor_tensor(
                            out=dst, in0=src,
                            in1=bias_sb["vb"][:, layer, :].rearrange(
                                "p (g h j) -> p g h j", g=3, h=2),
                            op=OP.add)
                    else:
                        nc.scalar.copy(dst, src)

                if DEBUG_L0 and layer == 0:
                    nc.sync.dma_start(dbg["va"][:, :], v_aug.rearrange("p a b -> p (a b)"))
                otc = per.tile([128, N_CHUNKS, N_TOK], MDT, tag="big")

                def emit_qk_chunks(pair):
                    """Returns (qkt dict, list of emission closures). Each
                    closure emits one n-tile psum group (3 matmuls + copies);
                    call all of them to finish QT/KT of this pair."""
                    qkt, chunks = {}, []
                    for nm, wd, bias_nm in (("q", d["wqp"], "qb"),
                                            ("k", d["wkp"], "kb")):
                        wqk = wts.tile([128, N_CHUNKS, 128], MDT, tag="wqk",
                                       bufs=2, name=f"wqk_{nm}")
                        for c in range(N_CHUNKS):
                            nc.sync.dma_start(
                                wqk[:, c, :],
                                wd[layer, pair, c * 128:(c + 1) * 128, :])
                        # bf16, both heads on partitions 0:64 (free axis = sub)
                        dstT = per.tile([64, 2, N_TOK], BF16, tag=nm + "t",
                                        bufs=1, name=f"qk_{nm}")
                        qkt[nm] = dstT

                        def chunk(n, wqk=wqk, dstT=dstT, bias_nm=bias_nm):
                            pq = psA.tile([128, 512], F32, tag="gen", name="pq")
                            for c in range(N_CHUNKS):
                                nc.tensor.matmul(
                                    pq, lhsT=_mm(wqk[:, c, :]),
                                    rhs=_mm(hT[:, c, n * 512:(n + 1) * 512]),
                                    start=(c == 0), stop=(c == N_CHUNKS - 1))
                            for sub in range(2):
                                dst = dstT[:, sub, n * 512:(n + 1) * 512]
                                src = pq[64 * sub:64 * sub + 64, :]
                                if has[bias_nm]:
                                    nc.scalar.activation(
                                        out=dst, in_=src, func=AF.Identity,
                                        bias=bias_sb[bias_nm][
                                            64 * sub:64 * sub + 64, layer,
                                            pair:pair + 1],
                                        scale=1.0)
                                elif sub == 0:
                                    nc.scalar.copy(dst, src)
                                else:
                                    nc.vector.tensor_copy(dst, src)

                        for n in range(N_TOK // 512):
                            chunks.append(lambda n=n, chunk=chunk: chunk(n))
                    return qkt, chunks

                for pair in range(3):
                    qkt, chunks = emit_qk_chunks(pair)
                    for ch in chunks:
                        ch()

                    # ---- attention, two (head, seq) units pipelined so the
                    # PE alternates units while ACT runs the exps ----
                    units = [(sub, s) for sub in range(2) for s in range(B_LOC)]
                    for g in range(0, len(units), 1):
                        pair_units = units[g:g + 1]
                        pos = {}
                        for u in pair_units:
                            pos[u] = psC.tile([128, 512], F32, tag="ot",
                                              name=f"po_{g}")
                        for ki in range(4):
                            width = 512 - ki * 128
                            # shared tile for both units: one batched mask op
                            at2 = wk.tile([128, 1, 512], MDT, tag="at_sb",
                                          bufs=4, name=f"at_{g}")
                            for j, u in enumerate(pair_units):
                                sub, s = u
                                kc = s * 512 + ki * 128
                                pa = psB.tile([128, 512], F32, tag="at",
                                              name=f"pa_{g}")
                                nc.tensor.matmul(
                                    pa[:, :width],
                                    lhsT=qkt["k"][:, sub, kc:kc + 128],
                                    rhs=qkt["q"][:, sub, kc:s * 512 + 512],
                                    start=True, stop=True)
                                nc.scalar.activation(
                                    out=at2[:, j, :width], in_=pa[:, :width],
                                    func=AF.Exp, scale=SCALE)
                                if DEBUG_L0 and layer == 0 and pair == 0 and u == (0, 0) and ki == 0:
                                    nc.sync.dma_start(dbg["at0"][:, :], at2[:, 0, :])
                            nu = len(pair_units)
                            nc.vector.tensor_tensor(
                                out=at2[:, :nu, 0:128], in0=at2[:, :nu, 0:128],
                                in1=tri[:, None, :].to_broadcast([128, nu, 128]),
                                op=OP.mult)
                            for j, u in enumerate(pair_units):
                                sub, s = u
                                off = V_OFF[2 * pair + sub]
                                nc.tensor.matmul(
                                    pos[u][:, ki * 128:512],
                                    lhsT=_mm(v_aug[:, s * 4 + ki, off:off + 128]),
                                    rhs=_mm(at2[:, j, :width]),
                                    start=(ki == 0), stop=(ki == 3))
                        for u in pair_units:
                            sub, s = u
                            po = pos[u]
                            ot_first = V_OT_FIRST[2 * pair + sub]
                            orow = 0 if ot_first else 64
                            rrow = 64 - orow
                            if DEBUG_L0 and layer == 0 and pair == 0 and u == (0, 0):
                                po_sb = wk.tile([128, 512], F32, tag="posb")
                                nc.scalar.copy(po_sb, po)
                                nc.sync.dma_start(dbg["po0"][:, :], po_sb)
                            rho = wk.tile([64, 512], F32, tag="rho", bufs=2,
                                          name=f"rho_{g}")
                            if not USE_FAST_RECIP:
                                nc.vector.reciprocal(
                                    out=rho, in_=po[rrow:rrow + 64, :])
                            elif rrow == 0:
                                # approx recip only works with base-0 input
                                nc.vector.reciprocal_approx_fast(
                                    out=rho, in_=po[0:64, :])
                            else:
                                r_sb = wk.tile([64, 512], F32, tag="rsb", bufs=2,
                                               name=f"rsb_{g}")
                                nc.scalar.copy(r_sb, po[rrow:rrow + 64, :])
                                nc.vector.reciprocal_approx_fast(
                                    out=rho, in_=r_sb)
                            nc.vector.tensor_tensor(
                                out=otc[64 * sub:64 * sub + 64, pair,
                                        s * 512:(s + 1) * 512],
                                in0=po[orow:orow + 64, :], in1=rho, op=OP.mult)
                            if DEBUG_L0 and layer == 0 and pair == 0 and u == (0, 0):
                                nc.sync.dma_start(dbg["rho0"][:, :], rho)

                if DEBUG_L0 and layer == 0:
                    nc.sync.dma_start(dbg["otc"][:, :], otc.rearrange("p a b -> p (a b)"))
                # ---- proj + residual ----
                wp_c = []
                for c in range(N_CHUNKS):
                    w = wts.tile([128, N_EMBED], MDT, tag="wchk", bufs=3)
                    nc.sync.dma_start(w, d["wp"][layer, c * 128:(c + 1) * 128, :])
                    wp_c.append(w)
                for t in range(N_TILES):
                    pp = psA.tile([128, 512], F32, tag="gen")
                    for c in range(N_CHUNKS):
                        nc.tensor.matmul(pp[:, :N_EMBED],
                                         lhsT=_mm(otc[:, c, t * 128:(t + 1) * 128]),
                                         rhs=_mm(wp_c[c]),
                                         start=(c == 0), stop=(c == N_CHUNKS - 1))
                    if has["bp"]:
                        tmp = hp.tile([128, N_EMBED], F32, tag="h")
                        nc.vector.tensor_tensor(out=tmp, in0=pp[:, :N_EMBED],
                                                in1=bias_sb["bp"][:, layer, :],
                                                op=OP.add)
                        nc.vector.tensor_tensor(out=x[:, t, :], in0=tmp,
                                                in1=x[:, t, :], op=OP.add)
                    else:
                        nc.vector.tensor_tensor(out=x[:, t, :], in0=pp[:, :N_EMBED],
                                                in1=x[:, t, :], op=OP.add)

                if DEBUG_L0 and layer == 0:
                    nc.sync.dma_start(dbg["x1"][:, :], x.rearrange("p a b -> p (a b)"))
                # ---- MLP ----
                h2T = per.tile([128, N_CHUNKS, N_TOK], MDT, tag="ht")
                layernorm_to(h2T, "ln2")
                w1all = wts.tile([128, N_CHUNKS, N_MLP], MDT, tag="w1all", bufs=1)
                for c in range(N_CHUNKS):
                    nc.sync.dma_start(
                        w1all[:, c, :], d["w1"][layer, c * 128:(c + 1) * 128, :])
                w2all = wts.tile([128, N_MCHUNK, N_EMBED], MDT, tag="w2all", bufs=1)
                for m in range(N_MCHUNK):
                    nc.sync.dma_start(
                        w2all[:, m, :], d["w2"][layer, m * 128:(m + 1) * 128, :])
                mlpT = per.tile([128, N_MCHUNK, 512], MDT, tag="big")
                for n in range(N_TOK // 512):
                    for m in range(N_MCHUNK):
                        pm = psA.tile([128, 512], F32, tag="gen")
                        for c in range(N_CHUNKS):
                            nc.tensor.matmul(
                                pm, lhsT=_mm(w1all[:, c, m * 128:(m + 1) * 128]),
                                rhs=_mm(h2T[:, c, n * 512:(n + 1) * 512]),
                                start=(c == 0), stop=(c == N_CHUNKS - 1))
                        if has["b1"]:
                            nc.scalar.activation(
                                out=mlpT[:, m, :], in_=pm, func=AF.Relu,
                                bias=bias_sb["b1"][:, layer, m:m + 1], scale=1.0)
                        else:
                            nc.scalar.activation(out=mlpT[:, m, :], in_=pm,
                                                 func=AF.Relu, scale=1.0)
                    for dt in range(4):
                        t = n * 4 + dt
                        pw = psA.tile([128, 512], F32, tag="gen")
                        for m in range(N_MCHUNK):
                            nc.tensor.matmul(
                                pw[:, :N_EMBED],
                                lhsT=_mm(mlpT[:, m, dt * 128:(dt + 1) * 128]),
                                rhs=_mm(w2all[:, m, :]),
                                start=(m == 0), stop=(m == N_MCHUNK - 1))
                        if has["b2"]:
                            tmp = hp.tile([128, N_EMBED], F32, tag="h")
                            nc.vector.tensor_tensor(out=tmp, in0=pw[:, :N_EMBED],
                                                    in1=bias_sb["b2"][:, layer, :],
                                                    op=OP.add)
                            nc.vector.tensor_tensor(out=x[:, t, :], in0=tmp,
                                                    in1=x[:, t, :], op=OP.add)
                        else:
                            nc.vector.tensor_tensor(out=x[:, t, :],
                                                    in0=pw[:, :N_EMBED],
                                                    in1=x[:, t, :], op=OP.add)

                if DEBUG_L0 and layer == 0:
                    nc.sync.dma_start(dbg["x2"][:, :], x.rearrange("p a b -> p (a b)"))
            # ---- final LN + LM head ----
            xfT = per.tile([128, N_CHUNKS, N_TOK], F32, tag="ht")
            layernorm_to(xfT, "lnf")
            wlm_c = []
            for c in range(N_CHUNKS):
                w = wts.tile([128, VOCAB], F32, tag="wlm", bufs=3)
                nc.sync.dma_start(w, d["wlm"][c * 128:(c + 1) * 128, :])
                wlm_c.append(w)
            for t in range(N_TILES):
                pl = psA.tile([128, 512], F32, tag="gen")
                for c in range(N_CHUNKS):
                    nc.tensor.matmul(pl[:, :VOCAB],
                                     lhsT=_mm(xfT[:, c, t * 128:(t + 1) * 128]),
                                     rhs=_mm(wlm_c[c]),
                                     start=(c == 0), stop=(c == N_CHUNKS - 1))
                lg = wk.tile([128, VOCAB], F32, tag="lg")
                if has["blm"]:
                    nc.vector.tensor_tensor(out=lg, in0=pl[:, :VOCAB],
                                            in1=bias_sb["blm"], op=OP.add)
                else:
                    nc.scalar.copy(lg, pl[:, :VOCAB])
                nc.sync.dma_start(logits_d[t * 128:(t + 1) * 128, :], lg)

    nc.compile()
    return nc


_CACHE = {}


def _get_nc(has):
    key = tuple(sorted(has.items()))
    if key not in _CACHE:
        _CACHE[key] = _build(has)
    return _CACHE[key]


def kernel(**inputs):
    shared, has, idx_f = _prep(inputs)
    nc = _get_nc(has)
    in_maps = []
    for core in range(N_CORES):
        m = dict(shared)
        m["idxf"] = idx_f[core]
        in_maps.append(m)
    res = run_bass_kernel_spmd(nc, in_maps, core_ids=list(range(N_CORES)))
    out = np.stack([r["logits"].reshape(B_LOC, T, VOCAB) for r in res.results])
    return out.reshape(B, T, VOCAB)



# revision 20
# speedup vs baseline: 1.1093x; 1.1093x over previous
"""Bass/Trainium2 kernel for a 6-layer GPT-style transformer (BigramLanguageModel).

Contract: kernel(**inputs) takes the FULL unsharded inputs from
reference.setup_inputs() and returns the FULL [32, 512, 65] fp32 logits.

Sharding: data-parallel over batch. Each of the 8 NeuronCores runs the whole
model on 4 of the 32 sequences (params replicated); outputs are concatenated
on the host. No collectives.

Device-side design (per core, 2048 tokens), v2 (all-bf16 matmuls):
 - all matmul operands bf16 (weights converted on host; activations written
   bf16 at the PSUM->SBUF copy). PSUM accumulation stays fp32. This enables
   FWL weight loads, 1 cyc/row matmuls everywhere, and 2x/4x DVE modes.
 - residual stream x: fp32 token-major SBUF [128, 16, 384].
 - LayerNorm: bn_stats/bn_aggr (DVE) in groups of 8 token tiles; apply writes
   bf16 h; PE transposes 8 tiles/chunk into one 2-bank PSUM tile; single
   [128,1024] DVE copy to the E-major hT buffer.
 - QKV: QT/KT per head-pair [128, 2048] bf16, head0 on partitions 0:64 and
   head1 on 64:128 (one PSUM->SBUF copy per 2 n-blocks); V token-major into
   v_aug [128, 16, 768] with per-head windows [ones64 | V_h64].
 - attention per (pair, seq): both heads' S^T blocks [128 k, width] computed
   into the two banks of one PSUM tile by row-packed concurrent K=64 matmuls;
   causal mask applied by PRE-ACCUMULATING -1e30 into the diagonal 128 cols
   via an ident@negtri matmul (start=True) so exp(scale*S) lands 0 exactly --
   no DVE mask op, chain is S(PE)->exp(ACT)->AV(PE). Batched exp over both
   heads [128, 2, width]. AV uses the [ones|V] stationary windows so PSUM
   rows 0:64 replicate the softmax denominator -> one batched fast-reciprocal
   + per-head tensor_tensor writes the normalized OT into the E-major concat
   buffer feeding the proj matmul.
 - MLP: mlpT = W1.T @ h2T (E-major), relu fused into the PSUM->SBUF copy
   (bf16), W2 with mlpT chunks stationary, token-major out + residual add.
 - logits: final LN -> xfT -> x @ Wlm per token tile, DMA out [2048, 65].
 - single PSUM pool of [128, 2, 512] (2-bank) tiles, bufs=4 = all 8 banks.
"""

import sys

for _p in ("/opt/trn_rl_repo", "/opt/pypackages"):
    if _p not in sys.path:
        sys.path.insert(0, _p)

import ml_dtypes
import numpy as np

import concourse.bass as bass
import concourse.tile as tile
from concourse import bacc, mybir
from concourse.bass_utils import run_bass_kernel_spmd

F32 = mybir.dt.float32
BF16 = mybir.dt.bfloat16

N_EMBED = 384
CONTEXT = 512
N_HEADS = 6
HEAD_DIM = 64
N_LAYERS = 6
VOCAB = 65
B, T = 32, 512
LN_EPS = 1e-5
N_CORES = 8
B_LOC = B // N_CORES          # 4 sequences per core
N_TOK = B_LOC * T             # 2048 tokens per core
N_TILES = N_TOK // 128        # 16 token tiles
N_CHUNKS = N_EMBED // 128     # 3 E-chunks
N_MLP = 4 * N_EMBED           # 1536
N_MCHUNK = N_MLP // 128       # 12
SCALE = float(N_EMBED) ** -0.5
# Mask addend: scale*NEG ~ -102 -> exp underflows to 0 (exact 0 after bf16
# cast). Huge magnitudes (-1e30) make the HW ACT exp LUT produce NaN.
NEG = -2000.0
DEBUG_L0 = False
V_W = N_HEADS * 128           # 768: per-head [ones64 | V64] windows


def _prep(inputs):
    """Host-side layout prep + exact LN folds. Returns (shared, has, idx)."""
    f = lambda a: np.ascontiguousarray(np.asarray(a), dtype=np.float32)
    bf = lambda a: np.ascontiguousarray(np.asarray(a)).astype(np.float32)
    idx = np.asarray(inputs["idx"])
    tok_emb, pos_emb = f(inputs["tok_emb"]), f(inputs["pos_emb"])
    Wq, Wk, Wv = f(inputs["Wq"]), f(inputs["Wk"]), f(inputs["Wv"])
    Wproj, bproj = f(inputs["Wproj"]), f(inputs["bproj"])
    W1, b1, W2, b2 = f(inputs["W1"]), f(inputs["b1"]), f(inputs["W2"]), f(inputs["b2"])
    ln1_g, ln1_b = f(inputs["ln1_g"]), f(inputs["ln1_b"])
    ln2_g, ln2_b = f(inputs["ln2_g"]), f(inputs["ln2_b"])
    lnf_g, lnf_b = f(inputs["lnf_g"]), f(inputs["lnf_b"])
    Wlm, blm = f(inputs["Wlm"]), f(inputs["blm"])

    L, H, E, D = N_LAYERS, N_HEADS, N_EMBED, HEAD_DIM

    # fold ln gains into the consuming weights (exact when g==1)
    Wq_f = ln1_g[:, None, :, None] * Wq          # [L,H,E,D]
    Wk_f = ln1_g[:, None, :, None] * Wk
    Wv_f = ln1_g[:, None, :, None] * Wv
    W1_f = ln2_g[:, :, None] * W1                # [L,E,4E]
    Wlm_f = lnf_g[:, None] * Wlm                 # [E,V]

    # ln biases propagate through the matmuls as constant bias vectors
    qb = np.einsum("le,lhed->lhd", ln1_b, Wq)    # [L,H,D]
    kb = np.einsum("le,lhed->lhd", ln1_b, Wk)
    vb = np.einsum("le,lhed->lhd", ln1_b, Wv)
    b1_eff = b1 + np.einsum("le,lem->lm", ln2_b, W1)    # [L,4E]
    blm_eff = blm + lnf_b @ Wlm                          # [V]

    # head-pair packed QT/KT weights: [L, 3, E, 128] (pair r = heads 2r, 2r+1)
    wqp = np.concatenate([Wq_f[:, 0::2], Wq_f[:, 1::2]], axis=-1)
    wkp = np.concatenate([Wk_f[:, 0::2], Wk_f[:, 1::2]], axis=-1)
    qbp = np.concatenate([qb[:, 0::2], qb[:, 1::2]], axis=-1)      # [L,3,128]
    kbp = np.concatenate([kb[:, 0::2], kb[:, 1::2]], axis=-1)
    wv_all = Wv_f.transpose(0, 2, 1, 3).reshape(L, E, H * D)       # [L,E,384]
    vb_all = vb.reshape(L, H * D)

    # negtri[k, q] = -1e30 where key k > query q (strict upper kept at 0)
    triu = np.triu(np.ones((128, 128), dtype=np.float32))
    negtri = (1.0 - triu) * NEG

    b16 = lambda a: np.ascontiguousarray(a).astype(ml_dtypes.bfloat16)
    shared = dict(
        tok_emb=b16(tok_emb),
        pos_emb=pos_emb,
        wqp=b16(wqp),
        wkp=b16(wkp),
        wv=b16(wv_all),
        wp=b16(Wproj),
        w1=b16(W1_f),
        w2=b16(W2),
        wlm=b16(Wlm_f),
        ident=b16(np.eye(128, dtype=np.float32)),
        iota=np.arange(VOCAB, dtype=np.float32).reshape(VOCAB, 1),
        negtri=b16(negtri),
    )
    flags = dict(
        qb=qbp if np.any(qbp) else None,
        kb=kbp if np.any(kbp) else None,
        vb=np.broadcast_to(vb_all[:, None, :], (L, 128, H * D)).copy()
        if np.any(vb) else None,
        bp=np.broadcast_to(bproj[:, None, :], (L, 128, E)).copy()
        if np.any(bproj) else None,
        b1=np.ascontiguousarray(b1_eff.reshape(L, N_MCHUNK, 128).transpose(0, 2, 1))
        if np.any(b1_eff) else None,                    # [L,128,12] partition-major
        b2=np.broadcast_to(b2[:, None, :], (L, 128, E)).copy() if np.any(b2) else None,
        blm=np.broadcast_to(blm_eff[None, :], (128, VOCAB)).copy()
        if np.any(blm_eff) else None,
    )
    for k, v in flags.items():
        if v is not None:
            shared[k] = np.ascontiguousarray(v, dtype=np.float32)
    has = {k: (v is not None) for k, v in flags.items()}

    idx_f = idx.astype(np.float32).reshape(N_CORES, N_TOK)
    return shared, has, idx_f


def _build(has):
    nc = bacc.Bacc(trn_type="TRN2", debug=False, num_devices=N_CORES)
    d = {}
    d["idxf"] = nc.dram_tensor("idxf", [N_TOK], F32, kind="ExternalInput")
    d["tok_emb"] = nc.dram_tensor("tok_emb", [VOCAB, N_EMBED], BF16, kind="ExternalInput")
    d["pos_emb"] = nc.dram_tensor("pos_emb", [CONTEXT, N_EMBED], F32, kind="ExternalInput")
    d["wqp"] = nc.dram_tensor("wqp", [N_LAYERS, 3, N_EMBED, 128], BF16, kind="ExternalInput")
    d["wkp"] = nc.dram_tensor("wkp", [N_LAYERS, 3, N_EMBED, 128], BF16, kind="ExternalInput")
    d["wv"] = nc.dram_tensor("wv", [N_LAYERS, N_EMBED, N_EMBED], BF16, kind="ExternalInput")
    d["wp"] = nc.dram_tensor("wp", [N_LAYERS, N_EMBED, N_EMBED], BF16, kind="ExternalInput")
    d["w1"] = nc.dram_tensor("w1", [N_LAYERS, N_EMBED, N_MLP], BF16, kind="ExternalInput")
    d["w2"] = nc.dram_tensor("w2", [N_LAYERS, N_MLP, N_EMBED], BF16, kind="ExternalInput")
    d["wlm"] = nc.dram_tensor("wlm", [N_EMBED, VOCAB], BF16, kind="ExternalInput")
    d["ident"] = nc.dram_tensor("ident", [128, 128], BF16, kind="ExternalInput")
    d["iota"] = nc.dram_tensor("iota", [VOCAB, 1], F32, kind="ExternalInput")
    d["negtri"] = nc.dram_tensor("negtri", [128, 128], BF16, kind="ExternalInput")
    if has["qb"]:
        d["qb"] = nc.dram_tensor("qb", [N_LAYERS, 3, 128], F32, kind="ExternalInput")
    if has["kb"]:
        d["kb"] = nc.dram_tensor("kb", [N_LAYERS, 3, 128], F32, kind="ExternalInput")
    if has["vb"]:
        d["vb"] = nc.dram_tensor("vb", [N_LAYERS, 128, N_EMBED], F32, kind="ExternalInput")
    if has["bp"]:
        d["bp"] = nc.dram_tensor("bp", [N_LAYERS, 128, N_EMBED], F32, kind="ExternalInput")
    if has["b1"]:
        d["b1"] = nc.dram_tensor("b1", [N_LAYERS, 128, N_MCHUNK], F32, kind="ExternalInput")
    if has["b2"]:
        d["b2"] = nc.dram_tensor("b2", [N_LAYERS, 128, N_EMBED], F32, kind="ExternalInput")
    if has["blm"]:
        d["blm"] = nc.dram_tensor("blm", [128, VOCAB], F32, kind="ExternalInput")
    logits_d = nc.dram_tensor("logits", [N_TOK, VOCAB], F32, kind="ExternalOutput")
    dbg = {}
    if DEBUG_L0:
        dbg["x0"] = nc.dram_tensor("dbg_x0", [128, N_TILES * N_EMBED], F32, kind="ExternalOutput")
        dbg["ht"] = nc.dram_tensor("dbg_ht", [128, N_CHUNKS * N_TOK], BF16, kind="ExternalOutput")
        dbg["qt"] = nc.dram_tensor("dbg_qt", [128, N_TOK], BF16, kind="ExternalOutput")
        dbg["kt"] = nc.dram_tensor("dbg_kt", [128, N_TOK], BF16, kind="ExternalOutput")
        dbg["va"] = nc.dram_tensor("dbg_va", [128, N_TILES * V_W], BF16, kind="ExternalOutput")
        dbg["at0"] = nc.dram_tensor("dbg_at0", [128, 2 * 512], BF16, kind="ExternalOutput")
        dbg["po0"] = nc.dram_tensor("dbg_po0", [128, 2 * 512], F32, kind="ExternalOutput")
        dbg["rho0"] = nc.dram_tensor("dbg_rho0", [64, 2 * 512], F32, kind="ExternalOutput")
        dbg["otc"] = nc.dram_tensor("dbg_otc", [128, N_CHUNKS * N_TOK], BF16, kind="ExternalOutput")
        dbg["x1"] = nc.dram_tensor("dbg_x1", [128, N_TILES * N_EMBED], F32, kind="ExternalOutput")

    AF = mybir.ActivationFunctionType
    OP = mybir.AluOpType

    with tile.TileContext(nc) as tc:
        with tc.tile_pool(name="const", bufs=1) as cst, \
             tc.tile_pool(name="persist", bufs=1) as per, \
             tc.tile_pool(name="work", bufs=3) as wk, \
             tc.tile_pool(name="htile", bufs=9) as hp, \
             tc.tile_pool(name="wts", bufs=4) as wts, \
             tc.tile_pool(name="ps", bufs=4, space="PSUM") as ps:

            # ---- constants ----
            ident = cst.tile([128, 128], BF16)
            nc.sync.dma_start(ident, d["ident"][:, :])
            iota = cst.tile([VOCAB, 1], F32)
            nc.sync.dma_start(iota, d["iota"][:, :])
            negtri = cst.tile([128, 128], BF16)
            nc.sync.dma_start(negtri, d["negtri"][:, :])
            eps_sb = cst.tile([128, 1], F32)
            nc.vector.memset(eps_sb, LN_EPS)
            tok_sb = cst.tile([VOCAB, N_EMBED], BF16)
            nc.sync.dma_start(tok_sb, d["tok_emb"][:, :])

            bias_sb = {}
            for nm, shp in (("vb", [128, N_EMBED]), ("bp", [128, N_EMBED]),
                            ("b2", [128, N_EMBED])):
                if has[nm]:
                    bias_sb[nm] = cst.tile([128, N_LAYERS, shp[1]], F32)
                    nc.sync.dma_start(
                        bias_sb[nm],
                        d[nm].rearrange("l p e -> p l e"))
            if has["b1"]:
                bias_sb["b1"] = cst.tile([128, N_LAYERS, N_MCHUNK], F32)
                nc.sync.dma_start(bias_sb["b1"], d["b1"].rearrange("l p m -> p l m"))
            for nm in ("qb", "kb"):
                if has[nm]:
                    bias_sb[nm] = cst.tile([128, N_LAYERS, 3], F32)
                    nc.sync.dma_start(bias_sb[nm], d[nm].rearrange("l r p -> p l r"))
            if has["blm"]:
                bias_sb["blm"] = cst.tile([128, VOCAB], F32)
                nc.sync.dma_start(bias_sb["blm"], d["blm"][:, :])

            # ---- persistent activations ----
            x = per.tile([128, N_TILES, N_EMBED], F32)          # residual, token-major
            v_aug = per.tile([128, N_TILES, V_W], BF16)         # per-head [ones|V]
            ones_blk = cst.tile([128, 64], BF16)
            nc.vector.memset(ones_blk, 1.0)
            for h in range(N_HEADS):                            # ones stripes
                nc.vector.tensor_copy(
                    v_aug[:, :, h * 128:h * 128 + 64],
                    ones_blk[:, None, :].to_broadcast([128, N_TILES, 64]))

            # ---- embedding: x = onehot(idx) @ tok_emb + pos ----
            for tp in range(N_TILES // 2):
                pe = ps.tile([128, 2, 512], F32, tag="ps")
                for dt in range(2):
                    t = tp * 2 + dt
                    idx_b = wk.tile([VOCAB, 128], F32, tag="idxb")
                    nc.sync.dma_start(
                        idx_b,
                        bass.AP(tensor=d["idxf"], offset=t * 128,
                                ap=[[0, VOCAB], [1, 128]]))
                    oh = wk.tile([VOCAB, 128], BF16, tag="oh")
                    nc.vector.tensor_scalar(out=oh, in0=idx_b, scalar1=iota,
                                            scalar2=None, op0=OP.is_equal)
                    nc.tensor.matmul(pe[:, dt, :N_EMBED], lhsT=oh, rhs=tok_sb,
                                     start=True, stop=True)
                    nc.scalar.copy(x[:, t, :], pe[:, dt, :N_EMBED])
                    nc.gpsimd.dma_start(
                        out=x[:, t, :],
                        in_=d["pos_emb"][(t % 4) * 128:(t % 4) * 128 + 128, :],
                        accum_op=OP.add)

            def layernorm_to(dst_hT, src_name):
                """bn_stats LN over x; writes E-major bf16 dst_hT [128, 3, 2048].

                Groups of 8 token tiles: stats -> rstd -> apply (bf16) ->
                per-chunk 8 PE transposes into a 2-bank PSUM tile -> one
                [128,1024] DVE copy out."""
                G = 8
                for tg in range(N_TILES // G):
                    mvg = wk.tile([128, G, 2], F32, tag="mv" + src_name)
                    for dt in range(G):
                        st = wk.tile([128, 6], F32, tag="bnst")
                        nc.vector.bn_stats(out=st, in_=x[:, tg * G + dt, :])
                        nc.vector.bn_aggr(out=mvg[:, dt, :], in_=st)
                    sstd = wk.tile([128, G], F32, tag="sstd")
                    nc.scalar.activation(out=sstd, in_=mvg[:, :, 1],
                                         func=AF.Sqrt, bias=eps_sb, scale=1.0)
                    rstd = wk.tile([128, G], F32, tag="rstd")
                    nc.vector.reciprocal(out=rstd, in_=sstd)
                    hts = []
                    for dt in range(G):
                        t = tg * G + dt
                        ht = hp.tile([128, N_EMBED], BF16, tag="h")
                        nc.vector.tensor_scalar(
                            out=ht, in0=x[:, t, :],
                            scalar1=mvg[:, dt, 0:1], scalar2=rstd[:, dt:dt + 1],
                            op0=OP.subtract, op1=OP.mult)
                        hts.append(ht)
                    # bf16 transposes: 8 x 128 cols = 1024 bf16 = one 2KB bank
                    pt = None
                    for c in range(N_CHUNKS):
                        if c % 2 == 0:
                            pt = ps.tile([128, 2, 1024], BF16, tag="ps")
                        bank = c % 2
                        for dt in range(G):
                            nc.tensor.transpose(
                                pt[:, bank, dt * 128:(dt + 1) * 128],
                                hts[dt][:, c * 128:(c + 1) * 128], ident)
                        nc.vector.tensor_copy(
                            dst_hT[:, c, tg * G * 128:(tg + 1) * G * 128],
                            pt[:, bank, :])

            for layer in range(N_LAYERS):
                if DEBUG_L0 and layer == 0:
                    nc.sync.dma_start(dbg["x0"][:, :], x.rearrange("p a b -> p (a b)"))
                hT = per.tile([128, N_CHUNKS, N_TOK], BF16, tag="ht1")
                layernorm_to(hT, "ln1")
                if DEBUG_L0 and layer == 0:
                    nc.sync.dma_start(dbg["ht"][:, :], hT.rearrange("p a b -> p (a b)"))

                # ---- V (token-major, into v_aug [ones|V] windows) ----
                wv_c = []
                for c in range(N_CHUNKS):
                    w = wts.tile([128, N_EMBED], BF16, tag="wchk", bufs=6)
                    nc.sync.dma_start(w, d["wv"][layer, c * 128:(c + 1) * 128, :])
                    wv_c.append(w)
                for tp in range(N_TILES // 2):
                    pv = ps.tile([128, 2, 512], F32, tag="ps")
                    for dt in range(2):
                        for c in range(N_CHUNKS):
                            nc.tensor.matmul(pv[:, dt, :N_EMBED],
                                             lhsT=hT[:, c, (tp * 2 + dt) * 128:
                                                     (tp * 2 + dt + 1) * 128],
                                             rhs=wv_c[c],
                                             start=(c == 0), stop=(c == N_CHUNKS - 1))
                    # scatter [128, 2, 6, 64] -> per-head V slots (offset 64)
                    src = pv[:, :, :N_EMBED].rearrange("p a (h j) -> p a h j", h=6)
                    dst = v_aug[:, tp * 2:tp * 2 + 2, :].rearrange(
                        "p a (h j) -> p a h j", h=6)[:, :, :, 64:128]
                    if has["vb"]:
                        nc.vector.tensor_tensor(
                            out=dst, in0=src,
                            in1=bias_sb["vb"][:, layer, :].rearrange(
                                "p (h j) -> p h j", h=6)[:, None, :, :]
                            .to_broadcast([128, 2, 6, 64]),
                            op=OP.add)
                    else:
                        nc.vector.tensor_copy(dst, src)

                if DEBUG_L0 and layer == 0:
                    nc.sync.dma_start(dbg["va"][:, :], v_aug.rearrange("p a b -> p (a b)"))
                otc = per.tile([128, N_CHUNKS, N_TOK], BF16, tag="otc")

                for pair in range(3):
                    # ---- QT/KT for this pair: [128, 2048] bf16,
                    # head0 on partitions 0:64, head1 on 64:128 ----
                    qkt = {}
                    for nm, wd, bias_nm in (("q", d["wqp"], "qb"),
                                            ("k", d["wkp"], "kb")):
                        wqk = wts.tile([128, N_CHUNKS, 128], BF16, tag="wqk",
                                       bufs=4, name=f"wqk_{nm}")
                        for c in range(N_CHUNKS):
                            nc.sync.dma_start(
                                wqk[:, c, :],
                                wd[layer, pair, c * 128:(c + 1) * 128, :])
                        dstT = per.tile([128, N_TOK], BF16, tag=nm + "t",
                                        bufs=2, name=f"qk_{nm}")
                        qkt[nm] = dstT
                        for np_ in range(N_TOK // 1024):
                            pq = ps.tile([128, 2, 512], F32, tag="ps", name="pq")
                            for half in range(2):
                                n = np_ * 2 + half
                                for c in range(N_CHUNKS):
                                    nc.tensor.matmul(
                                        pq[:, half, :],
                                        lhsT=wqk[:, c, :],
                                        rhs=hT[:, c, n * 512:(n + 1) * 512],
                                        start=(c == 0), stop=(c == N_CHUNKS - 1))
                            dst = dstT[:, np_ * 1024:(np_ + 1) * 1024]
                            if has[bias_nm]:
                                nc.scalar.activation(
                                    out=dst, in_=pq.rearrange("p a b -> p (a b)"),
                                    func=AF.Identity,
                                    bias=bias_sb[bias_nm][:, layer, pair:pair + 1],
                                    scale=1.0)
                            else:
                                nc.scalar.copy(
                                    dst, pq.rearrange("p a b -> p (a b)"))

                    # ---- attention: per seq, both heads together.
                    # S^T blocks for head j land in bank j of pa; the causal
                    # mask is pre-accumulated into the diagonal 128 cols. ----
                    for s in range(B_LOC):
                        po = ps.tile([128, 2, 512], F32, tag="ps", name="po")
                        for ki in range(4):
                            width = 512 - ki * 128
                            kc = s * 512 + ki * 128
                            pa = ps.tile([128, 2, 512], F32, tag="ps", name="pa")
                            at2 = wk.tile([128, 2, 512], BF16, tag="at_sb",
                                          bufs=4, name="at")
                            for j in range(2):
                                nc.tensor.matmul(
                                    pa[:, j, :width],
                                    lhsT=qkt["k"][j * 64:j * 64 + 64, kc:kc + 128],
                                    rhs=qkt["q"][j * 64:j * 64 + 64,
                                                 kc:s * 512 + 512],
                                    start=True, stop=False)
                            # accumulate -1e30 onto the diagonal 128 cols so
                            # exp(scale*S) lands exactly 0 where masked
                            for j in range(2):
                                nc.tensor.matmul(
                                    pa[:, j, 0:128], lhsT=ident, rhs=negtri,
                                    start=False, stop=True)
                            nc.scalar.activation(
                                out=at2[:, :, :width], in_=pa[:, :, :width],
                                func=AF.Exp, scale=SCALE)
                            if DEBUG_L0 and layer == 0 and pair == 0 and s == 0 and ki == 0:
                                nc.sync.dma_start(
                                    dbg["at0"][:, :],
                                    at2.rearrange("p a b -> p (a b)"))
                            for j in range(2):
                                h = 2 * pair + j
                                nc.tensor.matmul(
                                    po[:, j, ki * 128:512],
                                    lhsT=v_aug[:, s * 4 + ki,
                                               h * 128:h * 128 + 128],
                                    rhs=at2[:, j, :width],
                                    start=(ki == 0), stop=(ki == 3))
                        if DEBUG_L0 and layer == 0 and pair == 0 and s == 0:
                            po_sb = wk.tile([128, 2, 512], F32, tag="posb")
                            nc.vector.tensor_copy(po_sb, po)
                            nc.sync.dma_start(
                                dbg["po0"][:, :],
                                po_sb.rearrange("p a b -> p (a b)"))
                        # rows 0:64 of each bank replicate the denominator
                        rho = wk.tile([64, 2, 512], F32, tag="rho", bufs=2,
                                      name="rho")
                        nc.vector.reciprocal_approx_fast(
                            out=rho, in_=po[0:64, :, :])
                        if DEBUG_L0 and layer == 0 and pair == 0 and s == 0:
                            nc.sync.dma_start(
                                dbg["rho0"][:, :],
                                rho.rearrange("p a b -> p (a b)"))
                        for j in range(2):
                            nc.vector.tensor_tensor(
                                out=otc[64 * j:64 * j + 64, pair,
                                        s * 512:(s + 1) * 512],
                                in0=po[64:128, j, :], in1=rho[:, j, :],
                                op=OP.mult)

                if DEBUG_L0 and layer == 0:
                    nc.sync.dma_start(dbg["otc"][:, :], otc.rearrange("p a b -> p (a b)"))
                    for nm_, t_ in (("qt", qkt["q"]), ("kt", qkt["k"])):
                        nc.sync.dma_start(dbg[nm_][:, :], t_[:, :])
                # ---- proj + residual ----
                wp_c = []
                for c in range(N_CHUNKS):
                    w = wts.tile([128, N_EMBED], BF16, tag="wchk", bufs=6)
                    nc.sync.dma_start(w, d["wp"][layer, c * 128:(c + 1) * 128, :])
                    wp_c.append(w)
                for tp in range(N_TILES // 2):
                    pp = ps.tile([128, 2, 512], F32, tag="ps")
                    for dt in range(2):
                        for c in range(N_CHUNKS):
                            nc.tensor.matmul(
                                pp[:, dt, :N_EMBED],
                                lhsT=otc[:, c, (tp * 2 + dt) * 128:
                                         (tp * 2 + dt + 1) * 128],
                                rhs=wp_c[c],
                                start=(c == 0), stop=(c == N_CHUNKS - 1))
                    if has["bp"]:
                        tmp = hp.tile([128, 2, N_EMBED], F32, tag="tmp")
                        nc.vector.tensor_tensor(
                            out=tmp, in0=pp[:, :, :N_EMBED],
                            in1=bias_sb["bp"][:, None, layer, :]
                            .to_broadcast([128, 2, N_EMBED]), op=OP.add)
                        nc.vector.tensor_tensor(
                            out=x[:, tp * 2:tp * 2 + 2, :], in0=tmp,
                            in1=x[:, tp * 2:tp * 2 + 2, :], op=OP.add)
                    else:
                        nc.vector.tensor_tensor(
                            out=x[:, tp * 2:tp * 2 + 2, :],
                            in0=pp[:, :, :N_EMBED],
                            in1=x[:, tp * 2:tp * 2 + 2, :], op=OP.add)

                if DEBUG_L0 and layer == 0:
                    nc.sync.dma_start(dbg["x1"][:, :], x.rearrange("p a b -> p (a b)"))
                # ---- MLP ----
                h2T = per.tile([128, N_CHUNKS, N_TOK], BF16, tag="ht2")
                layernorm_to(h2T, "ln2")
                w1all = wts.tile([128, N_CHUNKS, N_MLP], BF16, tag="w1all", bufs=2)
                for c in range(N_CHUNKS):
                    nc.sync.dma_start(
                        w1all[:, c, :], d["w1"][layer, c * 128:(c + 1) * 128, :])
                w2all = wts.tile([128, N_MCHUNK, N_EMBED], BF16, tag="w2all", bufs=2)
                for m in range(N_MCHUNK):
                    nc.sync.dma_start(
                        w2all[:, m, :], d["w2"][layer, m * 128:(m + 1) * 128, :])
                mlpT = per.tile([128, N_MCHUNK, 512], BF16, tag="mlpt")
                for n in range(N_TOK // 512):
                    for mp in range(N_MCHUNK // 2):
                        pm = ps.tile([128, 2, 512], F32, tag="ps")
                        for dm in range(2):
                            m = mp * 2 + dm
                            for c in range(N_CHUNKS):
                                nc.tensor.matmul(
                                    pm[:, dm, :],
                                    lhsT=w1all[:, c, m * 128:(m + 1) * 128],
                                    rhs=h2T[:, c, n * 512:(n + 1) * 512],
                                    start=(c == 0), stop=(c == N_CHUNKS - 1))
                        if has["b1"]:
                            for dm in range(2):
                                nc.scalar.activation(
                                    out=mlpT[:, mp * 2 + dm, :],
                                    in_=pm[:, dm, :], func=AF.Relu,
                                    bias=bias_sb["b1"][:, layer,
                                                       mp * 2 + dm:mp * 2 + dm + 1],
                                    scale=1.0)
                        else:
                            nc.scalar.activation(
                                out=mlpT[:, mp * 2:mp * 2 + 2, :], in_=pm,
                                func=AF.Relu, scale=1.0)
                    for dp in range(2):
                        pw = ps.tile([128, 2, 512], F32, tag="ps")
                        for dt in range(2):
                            t = n * 4 + dp * 2 + dt
                            for m in range(N_MCHUNK):
                                nc.tensor.matmul(
                                    pw[:, dt, :N_EMBED],
                                    lhsT=mlpT[:, m, (dp * 2 + dt) * 128:
                                              (dp * 2 + dt + 1) * 128],
                                    rhs=w2all[:, m, :],
                                    start=(m == 0), stop=(m == N_MCHUNK - 1))
                        t0 = n * 4 + dp * 2
                        if has["b2"]:
                            tmp = hp.tile([128, 2, N_EMBED], F32, tag="tmp")
                            nc.vector.tensor_tensor(
                                out=tmp, in0=pw[:, :, :N_EMBED],
                                in1=bias_sb["b2"][:, None, layer, :]
                                .to_broadcast([128, 2, N_EMBED]), op=OP.add)
                            nc.vector.tensor_tensor(
                                out=x[:, t0:t0 + 2, :], in0=tmp,
                                in1=x[:, t0:t0 + 2, :], op=OP.add)
                        else:
                            nc.vector.tensor_tensor(
                                out=x[:, t0:t0 + 2, :],
                                in0=pw[:, :, :N_EMBED],
                                in1=x[:, t0:t0 + 2, :], op=OP.add)

            # ---- final LN + LM head ----
            xfT = per.tile([128, N_CHUNKS, N_TOK], BF16, tag="ht1")
            layernorm_to(xfT, "lnf")
            wlm_c = []
            for c in range(N_CHUNKS):
                w = wts.tile([128, VOCAB], BF16, tag="wlm", bufs=3)
                nc.sync.dma_start(w, d["wlm"][c * 128:(c + 1) * 128, :])
                wlm_c.append(w)
            for tp in range(N_TILES // 2):
                pl = ps.tile([128, 2, 512], F32, tag="ps")
                for dt in range(2):
                    for c in range(N_CHUNKS):
                        nc.tensor.matmul(
                            pl[:, dt, :VOCAB],
                            lhsT=xfT[:, c, (tp * 2 + dt) * 128:
                                     (tp * 2 + dt + 1) * 128],
                            rhs=wlm_c[c],
                            start=(c == 0), stop=(c == N_CHUNKS - 1))
                lg = wk.tile([128, 2, VOCAB], F32, tag="lg")
                if has["blm"]:
                    nc.vector.tensor_tensor(
                        out=lg, in0=pl[:, :, :VOCAB],
                        in1=bias_sb["blm"][:, None, :].to_broadcast(
                            [128, 2, VOCAB]), op=OP.add)
                else:
                    nc.vector.tensor_copy(lg, pl[:, :, :VOCAB])
                for dt in range(2):
                    t = tp * 2 + dt
                    nc.sync.dma_start(
                        logits_d[t * 128:(t + 1) * 128, :], lg[:, dt, :])

    nc.compile()
    return nc


_CACHE = {}


def _get_nc(has):
    key = tuple(sorted(has.items()))
    if key not in _CACHE:
        _CACHE[key] = _build(has)
    return _CACHE[key]


def kernel(**inputs):
    shared, has, idx_f = _prep(inputs)
    nc = _get_nc(has)
    in_maps = []
    for core in range(N_CORES):
        m = dict(shared)
        m["idxf"] = idx_f[core]
        in_maps.append(m)
    res = run_bass_kernel_spmd(nc, in_maps, core_ids=list(range(N_CORES)))
    out = np.stack([r["logits"].reshape(B_LOC, T, VOCAB) for r in res.results])
    return out.reshape(B, T, VOCAB)


# revision 32
# speedup vs baseline: 1.1438x; 1.0310x over previous
"""Bass/Trainium2 kernel for a 6-layer GPT-style transformer (BigramLanguageModel).

Contract: kernel(**inputs) takes the FULL unsharded inputs from
reference.setup_inputs() and returns the FULL [32, 512, 65] fp32 logits.

Sharding: data-parallel over batch. Each of the 8 NeuronCores runs the whole
model on 4 of the 32 sequences (params replicated); outputs are concatenated
on the host. No collectives.

Device-side design (per core, 2048 tokens), v2 (all-bf16 matmuls):
 - all matmul operands bf16 (weights converted on host; activations written
   bf16 at the PSUM->SBUF copy). PSUM accumulation stays fp32. This enables
   FWL weight loads, 1 cyc/row matmuls everywhere, and 2x/4x DVE modes.
 - residual stream x: fp32 token-major SBUF [128, 16, 384].
 - LayerNorm: bn_stats/bn_aggr (DVE) in groups of 8 token tiles; apply writes
   bf16 h; PE transposes 8 tiles/chunk into one 2-bank PSUM tile; single
   [128,1024] DVE copy to the E-major hT buffer.
 - QKV: QT/KT per head-pair [128, 2048] bf16, head0 on partitions 0:64 and
   head1 on 64:128 (one PSUM->SBUF copy per 2 n-blocks); V token-major into
   v_aug [128, 16, 768] with per-head windows [ones64 | V_h64].
 - attention per (pair, seq): both heads' S^T blocks [128 k, width] computed
   into the two banks of one PSUM tile by row-packed concurrent K=64 matmuls;
   causal mask applied by PRE-ACCUMULATING -1e30 into the diagonal 128 cols
   via an ident@negtri matmul (start=True) so exp(scale*S) lands 0 exactly --
   no DVE mask op, chain is S(PE)->exp(ACT)->AV(PE). Batched exp over both
   heads [128, 2, width]. AV uses the [ones|V] stationary windows so PSUM
   rows 0:64 replicate the softmax denominator -> one batched fast-reciprocal
   + per-head tensor_tensor writes the normalized OT into the E-major concat
   buffer feeding the proj matmul.
 - MLP: mlpT = W1.T @ h2T (E-major), relu fused into the PSUM->SBUF copy
   (bf16), W2 with mlpT chunks stationary, token-major out + residual add.
 - logits: final LN -> xfT -> x @ Wlm per token tile, DMA out [2048, 65].
 - single PSUM pool of [128, 2, 512] (2-bank) tiles, bufs=4 = all 8 banks.
"""

import sys

for _p in ("/opt/trn_rl_repo", "/opt/pypackages"):
    if _p not in sys.path:
        sys.path.insert(0, _p)

import ml_dtypes
import numpy as np

import concourse.bass as bass
import concourse.tile as tile
from concourse import bacc, mybir
from concourse.bass_utils import run_bass_kernel_spmd

F32 = mybir.dt.float32
BF16 = mybir.dt.bfloat16

N_EMBED = 384
CONTEXT = 512
N_HEADS = 6
HEAD_DIM = 64
N_LAYERS = 6
VOCAB = 65
B, T = 32, 512
LN_EPS = 1e-5
N_CORES = 8
B_LOC = B // N_CORES          # 4 sequences per core
N_TOK = B_LOC * T             # 2048 tokens per core
N_TILES = N_TOK // 128        # 16 token tiles
N_CHUNKS = N_EMBED // 128     # 3 E-chunks
N_MLP = 4 * N_EMBED           # 1536
N_MCHUNK = N_MLP // 128       # 12
SCALE = float(N_EMBED) ** -0.5
# Mask addend: scale*NEG ~ -102 -> exp underflows to 0 (exact 0 after bf16
# cast). Huge magnitudes (-1e30) make the HW ACT exp LUT produce NaN.
NEG = -2000.0
DEBUG_L0 = False
V_W = N_HEADS * 128           # 768: per-head [ones64 | V64] windows


def _prep(inputs):
    """Host-side layout prep + exact LN folds. Returns (shared, has, idx)."""
    f = lambda a: np.ascontiguousarray(np.asarray(a), dtype=np.float32)
    bf = lambda a: np.ascontiguousarray(np.asarray(a)).astype(np.float32)
    idx = np.asarray(inputs["idx"])
    tok_emb, pos_emb = f(inputs["tok_emb"]), f(inputs["pos_emb"])
    Wq, Wk, Wv = f(inputs["Wq"]), f(inputs["Wk"]), f(inputs["Wv"])
    Wproj, bproj = f(inputs["Wproj"]), f(inputs["bproj"])
    W1, b1, W2, b2 = f(inputs["W1"]), f(inputs["b1"]), f(inputs["W2"]), f(inputs["b2"])
    ln1_g, ln1_b = f(inputs["ln1_g"]), f(inputs["ln1_b"])
    ln2_g, ln2_b = f(inputs["ln2_g"]), f(inputs["ln2_b"])
    lnf_g, lnf_b = f(inputs["lnf_g"]), f(inputs["lnf_b"])
    Wlm, blm = f(inputs["Wlm"]), f(inputs["blm"])

    L, H, E, D = N_LAYERS, N_HEADS, N_EMBED, HEAD_DIM

    # fold ln gains into the consuming weights (exact when g==1)
    Wq_f = ln1_g[:, None, :, None] * Wq          # [L,H,E,D]
    Wk_f = ln1_g[:, None, :, None] * Wk
    Wv_f = ln1_g[:, None, :, None] * Wv
    W1_f = ln2_g[:, :, None] * W1                # [L,E,4E]
    Wlm_f = lnf_g[:, None] * Wlm                 # [E,V]

    # ln biases propagate through the matmuls as constant bias vectors
    qb = np.einsum("le,lhed->lhd", ln1_b, Wq)    # [L,H,D]
    kb = np.einsum("le,lhed->lhd", ln1_b, Wk)
    vb = np.einsum("le,lhed->lhd", ln1_b, Wv)
    b1_eff = b1 + np.einsum("le,lem->lm", ln2_b, W1)    # [L,4E]
    blm_eff = blm + lnf_b @ Wlm                          # [V]

    # head-pair packed QT/KT weights: [L, 3, E, 128] (pair r = heads 2r, 2r+1)
    wqp = np.concatenate([Wq_f[:, 0::2], Wq_f[:, 1::2]], axis=-1)
    wkp = np.concatenate([Wk_f[:, 0::2], Wk_f[:, 1::2]], axis=-1)
    qbp = np.concatenate([qb[:, 0::2], qb[:, 1::2]], axis=-1)      # [L,3,128]
    kbp = np.concatenate([kb[:, 0::2], kb[:, 1::2]], axis=-1)
    wv_all = Wv_f.transpose(0, 2, 1, 3).reshape(L, E, H * D)       # [L,E,384]
    vb_all = vb.reshape(L, H * D)

    # negtri[k, q] = -1e30 where key k > query q (strict upper kept at 0)
    triu = np.triu(np.ones((128, 128), dtype=np.float32))
    negtri = (1.0 - triu) * NEG

    b16 = lambda a: np.ascontiguousarray(a).astype(ml_dtypes.bfloat16)
    shared = dict(
        tok_emb=b16(tok_emb),
        pos_emb=b16(pos_emb),
        wqp=b16(wqp),
        wkp=b16(wkp),
        wv=b16(wv_all),
        wp=b16(Wproj),
        w1=b16(W1_f),
        w2=b16(W2),
        wlm=b16(Wlm_f),
        ident=b16(np.eye(128, dtype=np.float32)),
        iota=np.arange(VOCAB, dtype=np.float32).reshape(VOCAB, 1),
        negtri=b16(negtri),
    )
    flags = dict(
        qb=qbp if np.any(qbp) else None,
        kb=kbp if np.any(kbp) else None,
        vb=np.broadcast_to(vb_all[:, None, :], (L, 128, H * D)).copy()
        if np.any(vb) else None,
        bp=np.broadcast_to(bproj[:, None, :], (L, 128, E)).copy()
        if np.any(bproj) else None,
        b1=np.ascontiguousarray(b1_eff.reshape(L, N_MCHUNK, 128).transpose(0, 2, 1))
        if np.any(b1_eff) else None,                    # [L,128,12] partition-major
        b2=np.broadcast_to(b2[:, None, :], (L, 128, E)).copy() if np.any(b2) else None,
        blm=np.broadcast_to(blm_eff[None, :], (128, VOCAB)).copy()
        if np.any(blm_eff) else None,
    )
    for k, v in flags.items():
        if v is not None:
            shared[k] = np.ascontiguousarray(v, dtype=np.float32)
    has = {k: (v is not None) for k, v in flags.items()}

    idx_f = idx.astype(np.float32).reshape(N_CORES, N_TOK)
    return shared, has, idx_f


def _build(has):
    nc = bacc.Bacc(trn_type="TRN2", debug=False, num_devices=N_CORES)
    d = {}
    d["idxf"] = nc.dram_tensor("idxf", [N_TOK], F32, kind="ExternalInput")
    d["tok_emb"] = nc.dram_tensor("tok_emb", [VOCAB, N_EMBED], BF16, kind="ExternalInput")
    d["pos_emb"] = nc.dram_tensor("pos_emb", [CONTEXT, N_EMBED], BF16, kind="ExternalInput")
    d["wqp"] = nc.dram_tensor("wqp", [N_LAYERS, 3, N_EMBED, 128], BF16, kind="ExternalInput")
    d["wkp"] = nc.dram_tensor("wkp", [N_LAYERS, 3, N_EMBED, 128], BF16, kind="ExternalInput")
    d["wv"] = nc.dram_tensor("wv", [N_LAYERS, N_EMBED, N_EMBED], BF16, kind="ExternalInput")
    d["wp"] = nc.dram_tensor("wp", [N_LAYERS, N_EMBED, N_EMBED], BF16, kind="ExternalInput")
    d["w1"] = nc.dram_tensor("w1", [N_LAYERS, N_EMBED, N_MLP], BF16, kind="ExternalInput")
    d["w2"] = nc.dram_tensor("w2", [N_LAYERS, N_MLP, N_EMBED], BF16, kind="ExternalInput")
    d["wlm"] = nc.dram_tensor("wlm", [N_EMBED, VOCAB], BF16, kind="ExternalInput")
    d["ident"] = nc.dram_tensor("ident", [128, 128], BF16, kind="ExternalInput")
    d["iota"] = nc.dram_tensor("iota", [VOCAB, 1], F32, kind="ExternalInput")
    d["negtri"] = nc.dram_tensor("negtri", [128, 128], BF16, kind="ExternalInput")
    if has["qb"]:
        d["qb"] = nc.dram_tensor("qb", [N_LAYERS, 3, 128], F32, kind="ExternalInput")
    if has["kb"]:
        d["kb"] = nc.dram_tensor("kb", [N_LAYERS, 3, 128], F32, kind="ExternalInput")
    if has["vb"]:
        d["vb"] = nc.dram_tensor("vb", [N_LAYERS, 128, N_EMBED], F32, kind="ExternalInput")
    if has["bp"]:
        d["bp"] = nc.dram_tensor("bp", [N_LAYERS, 128, N_EMBED], F32, kind="ExternalInput")
    if has["b1"]:
        d["b1"] = nc.dram_tensor("b1", [N_LAYERS, 128, N_MCHUNK], F32, kind="ExternalInput")
    if has["b2"]:
        d["b2"] = nc.dram_tensor("b2", [N_LAYERS, 128, N_EMBED], F32, kind="ExternalInput")
    if has["blm"]:
        d["blm"] = nc.dram_tensor("blm", [128, VOCAB], F32, kind="ExternalInput")
    logits_d = nc.dram_tensor("logits", [N_TOK, VOCAB], F32, kind="ExternalOutput")
    dbg = {}
    if DEBUG_L0:
        dbg["x0"] = nc.dram_tensor("dbg_x0", [128, N_TILES * N_EMBED], BF16, kind="ExternalOutput")
        dbg["ht"] = nc.dram_tensor("dbg_ht", [128, N_CHUNKS * N_TOK], BF16, kind="ExternalOutput")
        dbg["qt"] = nc.dram_tensor("dbg_qt", [128, N_TOK], BF16, kind="ExternalOutput")
        dbg["kt"] = nc.dram_tensor("dbg_kt", [128, N_TOK], BF16, kind="ExternalOutput")
        dbg["va"] = nc.dram_tensor("dbg_va", [128, N_TILES * V_W], BF16, kind="ExternalOutput")
        dbg["at0"] = nc.dram_tensor("dbg_at0", [128, 2 * 512], BF16, kind="ExternalOutput")
        dbg["po0"] = nc.dram_tensor("dbg_po0", [128, 2 * 512], F32, kind="ExternalOutput")
        dbg["rho0"] = nc.dram_tensor("dbg_rho0", [64, 2 * 512], F32, kind="ExternalOutput")
        dbg["otc"] = nc.dram_tensor("dbg_otc", [128, N_CHUNKS * N_TOK], BF16, kind="ExternalOutput")
        dbg["x1"] = nc.dram_tensor("dbg_x1", [128, N_TILES * N_EMBED], BF16, kind="ExternalOutput")

    AF = mybir.ActivationFunctionType
    OP = mybir.AluOpType

    with tile.TileContext(nc) as tc:
        with tc.tile_pool(name="const", bufs=1) as cst, \
             tc.tile_pool(name="persist", bufs=1) as per, \
             tc.tile_pool(name="work", bufs=3) as wk, \
             tc.tile_pool(name="htile", bufs=9) as hp, \
             tc.tile_pool(name="wts", bufs=4) as wts, \
             tc.tile_pool(name="ps", bufs=4, space="PSUM") as ps:

            # ---- constants ----
            ident = cst.tile([128, 128], BF16)
            nc.sync.dma_start(ident, d["ident"][:, :])
            iota = cst.tile([VOCAB, 1], F32)
            nc.sync.dma_start(iota, d["iota"][:, :])
            negtri = cst.tile([128, 128], BF16)
            nc.sync.dma_start(negtri, d["negtri"][:, :])
            eps_sb = cst.tile([128, 1], F32)
            nc.vector.memset(eps_sb, LN_EPS)
            tok_sb = cst.tile([VOCAB, N_EMBED], BF16)
            nc.sync.dma_start(tok_sb, d["tok_emb"][:, :])

            bias_sb = {}
            for nm, shp in (("vb", [128, N_EMBED]), ("bp", [128, N_EMBED]),
                            ("b2", [128, N_EMBED])):
                if has[nm]:
                    bias_sb[nm] = cst.tile([128, N_LAYERS, shp[1]], F32)
                    nc.sync.dma_start(
                        bias_sb[nm],
                        d[nm].rearrange("l p e -> p l e"))
            if has["b1"]:
                bias_sb["b1"] = cst.tile([128, N_LAYERS, N_MCHUNK], F32)
                nc.sync.dma_start(bias_sb["b1"], d["b1"].rearrange("l p m -> p l m"))
            for nm in ("qb", "kb"):
                if has[nm]:
                    bias_sb[nm] = cst.tile([128, N_LAYERS, 3], F32)
                    nc.sync.dma_start(bias_sb[nm], d[nm].rearrange("l r p -> p l r"))
            if has["blm"]:
                bias_sb["blm"] = cst.tile([128, VOCAB], F32)
                nc.sync.dma_start(bias_sb["blm"], d["blm"][:, :])

            # ---- persistent activations ----
            x = per.tile([128, N_TILES, N_EMBED], BF16)         # residual, token-major
            pos_sb = cst.tile([128, B_LOC, N_EMBED], BF16)
            nc.sync.dma_start(
                pos_sb, d["pos_emb"].rearrange("(a p) e -> p a e", p=128))
            v_aug = per.tile([128, N_TILES, V_W], BF16)         # per-head [ones|V]
            ones_blk = cst.tile([128, 64], BF16)
            nc.vector.memset(ones_blk, 1.0)
            for h in range(N_HEADS):                            # ones stripes
                nc.vector.tensor_copy(
                    v_aug[:, :, h * 128:h * 128 + 64],
                    ones_blk[:, None, :].to_broadcast([128, N_TILES, 64]))

            # ---- embedding: x = onehot(idx) @ tok_emb + pos ----
            for tp in range(N_TILES // 2):
                pe = ps.tile([128, 2, 512], F32, tag="ps")
                for dt in range(2):
                    t = tp * 2 + dt
                    idx_b = wk.tile([VOCAB, 128], F32, tag="idxb")
                    nc.sync.dma_start(
                        idx_b,
                        bass.AP(tensor=d["idxf"], offset=t * 128,
                                ap=[[0, VOCAB], [1, 128]]))
                    oh = wk.tile([VOCAB, 128], BF16, tag="oh")
                    nc.vector.tensor_scalar(out=oh, in0=idx_b, scalar1=iota,
                                            scalar2=None, op0=OP.is_equal)
                    nc.tensor.matmul(pe[:, dt, :N_EMBED], lhsT=oh, rhs=tok_sb,
                                     start=True, stop=False)
                    nc.tensor.matmul(pe[:, dt, :N_EMBED], lhsT=ident,
                                     rhs=pos_sb[:, t % B_LOC, :],
                                     start=False, stop=True)
                    nc.scalar.copy(x[:, t, :], pe[:, dt, :N_EMBED])

            def layernorm_to(dst_hT, src_name):
                """bn_stats LN over x; writes E-major bf16 dst_hT [128, 3, 2048].

                Groups of 4 token tiles (short pre-transpose latency): stats ->
                rstd -> apply (bf16) -> 12 PE transposes into one 2-bank bf16
                PSUM tile (chunks 0,1 in bank0, chunk 2 in bank1) -> 2 DVE
                copies out."""
                G = 4
                for tg in range(N_TILES // G):
                    mvg = wk.tile([128, G, 2], F32, tag="mv" + src_name)
                    for dt in range(G):
                        st = wk.tile([128, 6], F32, tag="bnst")
                        nc.vector.bn_stats(out=st, in_=x[:, tg * G + dt, :])
                        nc.vector.bn_aggr(out=mvg[:, dt, :], in_=st)
                    sstd = wk.tile([128, G], F32, tag="sstd")
                    nc.scalar.activation(out=sstd, in_=mvg[:, :, 1],
                                         func=AF.Sqrt, bias=eps_sb, scale=1.0)
                    rstd = wk.tile([128, G], F32, tag="rstd")
                    nc.vector.reciprocal(out=rstd, in_=sstd)
                    hts = []
                    for dt in range(G):
                        t = tg * G + dt
                        ht = hp.tile([128, N_EMBED], BF16, tag="h")
                        nc.vector.tensor_scalar(
                            out=ht, in0=x[:, t, :],
                            scalar1=mvg[:, dt, 0:1], scalar2=rstd[:, dt:dt + 1],
                            op0=OP.subtract, op1=OP.mult)
                        hts.append(ht)
                    pt = ps.tile([128, 2, 1024], BF16, tag="ps")
                    for c in range(N_CHUNKS):
                        for dt in range(G):
                            nc.tensor.transpose(
                                pt[:, c // 2, (c % 2) * 512 + dt * 128:
                                   (c % 2) * 512 + (dt + 1) * 128],
                                hts[dt][:, c * 128:(c + 1) * 128], ident)
                    nc.vector.tensor_copy(
                        dst_hT[:, 0:2, tg * G * 128:(tg + 1) * G * 128],
                        pt[:, 0, :].rearrange("p (a b) -> p a b", a=2))
                    nc.vector.tensor_copy(
                        dst_hT[:, 2, tg * G * 128:(tg + 1) * G * 128],
                        pt[:, 1, 0:512])

            for layer in range(N_LAYERS):
                if DEBUG_L0 and layer == 0:
                    nc.sync.dma_start(dbg["x0"][:, :], x.rearrange("p a b -> p (a b)"))
                hT = per.tile([128, N_CHUNKS, N_TOK], BF16, tag="ht1")
                layernorm_to(hT, "ln1")
                if DEBUG_L0 and layer == 0:
                    nc.sync.dma_start(dbg["ht"][:, :], hT.rearrange("p a b -> p (a b)"))

                # ---- V (token-major, into v_aug [ones|V] windows) ----
                wv_c = []
                for c in range(N_CHUNKS):
                    w = wts.tile([128, N_EMBED], BF16, tag="wchk", bufs=6)
                    nc.sync.dma_start(w, d["wv"][layer, c * 128:(c + 1) * 128, :])
                    wv_c.append(w)
                for tp in range(N_TILES // 2):
                    pv = ps.tile([128, 2, 512], F32, tag="ps")
                    for dt in range(2):
                        for c in range(N_CHUNKS):
                            nc.tensor.matmul(pv[:, dt, :N_EMBED],
                                             lhsT=hT[:, c, (tp * 2 + dt) * 128:
                                                     (tp * 2 + dt + 1) * 128],
                                             rhs=wv_c[c],
                                             start=(c == 0), stop=(c == N_CHUNKS - 1))
                    # scatter [128, 2, 6, 64] -> per-head V slots (offset 64)
                    src = pv[:, :, :N_EMBED].rearrange("p a (h j) -> p a h j", h=6)
                    dst = v_aug[:, tp * 2:tp * 2 + 2, :].rearrange(
                        "p a (h j) -> p a h j", h=6)[:, :, :, 64:128]
                    if has["vb"]:
                        nc.vector.tensor_tensor(
                            out=dst, in0=src,
                            in1=bias_sb["vb"][:, layer, :].rearrange(
                                "p (h j) -> p h j", h=6)[:, None, :, :]
                            .to_broadcast([128, 2, 6, 64]),
                            op=OP.add)
                    else:
                        nc.vector.tensor_copy(dst, src)

                if DEBUG_L0 and layer == 0:
                    nc.sync.dma_start(dbg["va"][:, :], v_aug.rearrange("p a b -> p (a b)"))
                otc = per.tile([128, N_CHUNKS, N_TOK], BF16, tag="otc")

                def emit_qk(pair):
                    # ---- QT/KT for this pair: [128, 2048] bf16,
                    # head0 on partitions 0:64, head1 on 64:128 ----
                    qkt = {}
                    for nm, wd, bias_nm in (("q", d["wqp"], "qb"),
                                            ("k", d["wkp"], "kb")):
                        wqk = wts.tile([128, N_CHUNKS, 128], BF16, tag="wqk",
                                       bufs=4, name=f"wqk_{nm}")
                        for c in range(N_CHUNKS):
                            nc.sync.dma_start(
                                wqk[:, c, :],
                                wd[layer, pair, c * 128:(c + 1) * 128, :])
                        dstT = per.tile([128, N_TOK], BF16, tag=nm + "t",
                                        bufs=2, name=f"qk_{nm}")
                        qkt[nm] = dstT
                        for np_ in range(N_TOK // 1024):
                            pq = ps.tile([128, 2, 512], F32, tag="ps", name="pq")
                            for half in range(2):
                                n = np_ * 2 + half
                                for c in range(N_CHUNKS):
                                    nc.tensor.matmul(
                                        pq[:, half, :],
                                        lhsT=wqk[:, c, :],
                                        rhs=hT[:, c, n * 512:(n + 1) * 512],
                                        start=(c == 0), stop=(c == N_CHUNKS - 1))
                            dst = dstT[:, np_ * 1024:(np_ + 1) * 1024]
                            if has[bias_nm]:
                                nc.scalar.activation(
                                    out=dst, in_=pq.rearrange("p a b -> p (a b)"),
                                    func=AF.Identity,
                                    bias=bias_sb[bias_nm][:, layer, pair:pair + 1],
                                    scale=1.0)
                            else:
                                nc.scalar.copy(
                                    dst, pq.rearrange("p a b -> p (a b)"))
                    return qkt

                def emit_att(pair, qkt):
                    # ---- attention: per seq, both heads together.
                    # S^T blocks for head j land in bank j of pa; the causal
                    # mask is pre-accumulated into the diagonal 128 cols. ----
                    for s in range(B_LOC):
                        po = ps.tile([128, 2, 512], F32, tag="ps", name="po")
                        for ki in range(4):
                            width = 512 - ki * 128
                            kc = s * 512 + ki * 128
                            pa = ps.tile([128, 2, 512], F32, tag="ps", name="pa")
                            at2 = wk.tile([128, 2, 512], BF16, tag="at_sb",
                                          bufs=4, name="at")
                            for j in range(2):
                                nc.tensor.matmul(
                                    pa[:, j, :width],
                                    lhsT=qkt["k"][j * 64:j * 64 + 64, kc:kc + 128],
                                    rhs=qkt["q"][j * 64:j * 64 + 64,
                                                 kc:s * 512 + 512],
                                    start=True, stop=False)
                            # accumulate -1e30 onto the diagonal 128 cols so
                            # exp(scale*S) lands exactly 0 where masked
                            for j in range(2):
                                nc.tensor.matmul(
                                    pa[:, j, 0:128], lhsT=ident, rhs=negtri,
                                    start=False, stop=True)
                            nc.scalar.activation(
                                out=at2[:, :, :width], in_=pa[:, :, :width],
                                func=AF.Exp, scale=SCALE)
                            if DEBUG_L0 and layer == 0 and pair == 0 and s == 0 and ki == 0:
                                nc.sync.dma_start(
                                    dbg["at0"][:, :],
                                    at2.rearrange("p a b -> p (a b)"))
                            for j in range(2):
                                h = 2 * pair + j
                                nc.tensor.matmul(
                                    po[:, j, ki * 128:512],
                                    lhsT=v_aug[:, s * 4 + ki,
                                               h * 128:h * 128 + 128],
                                    rhs=at2[:, j, :width],
                                    start=(ki == 0), stop=(ki == 3))
                        if DEBUG_L0 and layer == 0 and pair == 0 and s == 0:
                            po_sb = wk.tile([128, 2, 512], F32, tag="posb")
                            nc.vector.tensor_copy(po_sb, po)
                            nc.sync.dma_start(
                                dbg["po0"][:, :],
                                po_sb.rearrange("p a b -> p (a b)"))
                        # rows 0:64 of each bank replicate the denominator
                        rho = wk.tile([64, 2, 512], F32, tag="rho", bufs=2,
                                      name="rho")
                        nc.vector.reciprocal_approx_fast(
                            out=rho, in_=po[0:64, :, :])
                        if DEBUG_L0 and layer == 0 and pair == 0 and s == 0:
                            nc.sync.dma_start(
                                dbg["rho0"][:, :],
                                rho.rearrange("p a b -> p (a b)"))
                        for j in range(2):
                            nc.vector.tensor_tensor(
                                out=otc[64 * j:64 * j + 64, pair,
                                        s * 512:(s + 1) * 512],
                                in0=po[64:128, j, :], in1=rho[:, j, :],
                                op=OP.mult)

                # QK of pair p+1 is emitted before attention of pair p so the
                # PE has dense matmul work to hide the exp (ACT) latency
                qk_cache = {0: emit_qk(0), 1: emit_qk(1)}
                emit_att(0, qk_cache[0])
                qk_cache[2] = emit_qk(2)
                emit_att(1, qk_cache[1])
                emit_att(2, qk_cache[2])

                if DEBUG_L0 and layer == 0:
                    nc.sync.dma_start(dbg["otc"][:, :], otc.rearrange("p a b -> p (a b)"))
                    for nm_, t_ in (("qt", qk_cache[2]["q"]), ("kt", qk_cache[2]["k"])):
                        nc.sync.dma_start(dbg[nm_][:, :], t_[:, :])
                # ---- proj + residual ----
                wp_c = []
                for c in range(N_CHUNKS):
                    w = wts.tile([128, N_EMBED], BF16, tag="wchk", bufs=6)
                    nc.sync.dma_start(w, d["wp"][layer, c * 128:(c + 1) * 128, :])
                    wp_c.append(w)
                for tp in range(N_TILES // 2):
                    pp = ps.tile([128, 2, 512], F32, tag="ps")
                    for dt in range(2):
                        t = tp * 2 + dt
                        for c in range(N_CHUNKS):
                            nc.tensor.matmul(
                                pp[:, dt, :N_EMBED],
                                lhsT=otc[:, c, t * 128:(t + 1) * 128],
                                rhs=wp_c[c],
                                start=(c == 0), stop=False)
                        # residual: accumulate x into PSUM via identity matmul
                        nc.tensor.matmul(pp[:, dt, :N_EMBED], lhsT=ident,
                                         rhs=x[:, t, :], start=False, stop=True)
                    if has["bp"]:
                        nc.vector.tensor_tensor(
                            out=x[:, tp * 2:tp * 2 + 2, :],
                            in0=pp[:, :, :N_EMBED],
                            in1=bias_sb["bp"][:, None, layer, :]
                            .to_broadcast([128, 2, N_EMBED]), op=OP.add)
                    else:
                        nc.scalar.copy(x[:, tp * 2:tp * 2 + 2, :],
                                       pp[:, :, :N_EMBED])

                if DEBUG_L0 and layer == 0:
                    nc.sync.dma_start(dbg["x1"][:, :], x.rearrange("p a b -> p (a b)"))
                # ---- MLP ----
                h2T = per.tile([128, N_CHUNKS, N_TOK], BF16, tag="ht2")
                layernorm_to(h2T, "ln2")
                w1all = wts.tile([128, N_CHUNKS, N_MLP], BF16, tag="w1all", bufs=2)
                for c in range(N_CHUNKS):
                    nc.sync.dma_start(
                        w1all[:, c, :], d["w1"][layer, c * 128:(c + 1) * 128, :])
                w2all = wts.tile([128, N_MCHUNK, N_EMBED], BF16, tag="w2all", bufs=2)
                for m in range(N_MCHUNK):
                    nc.sync.dma_start(
                        w2all[:, m, :], d["w2"][layer, m * 128:(m + 1) * 128, :])
                mlpT = per.tile([128, N_MCHUNK, 512], BF16, tag="mlpt")
                for n in range(N_TOK // 512):
                    for mp in range(N_MCHUNK // 2):
                        pm = ps.tile([128, 2, 512], F32, tag="ps")
                        for dm in range(2):
                            m = mp * 2 + dm
                            for c in range(N_CHUNKS):
                                nc.tensor.matmul(
                                    pm[:, dm, :],
                                    lhsT=w1all[:, c, m * 128:(m + 1) * 128],
                                    rhs=h2T[:, c, n * 512:(n + 1) * 512],
                                    start=(c == 0), stop=(c == N_CHUNKS - 1))
                        if has["b1"]:
                            for dm in range(2):
                                nc.scalar.activation(
                                    out=mlpT[:, mp * 2 + dm, :],
                                    in_=pm[:, dm, :], func=AF.Relu,
                                    bias=bias_sb["b1"][:, layer,
                                                       mp * 2 + dm:mp * 2 + dm + 1],
                                    scale=1.0)
                        else:
                            nc.scalar.activation(
                                out=mlpT[:, mp * 2:mp * 2 + 2, :], in_=pm,
                                func=AF.Relu, scale=1.0)
                    for dp in range(2):
                        pw = ps.tile([128, 2, 512], F32, tag="ps")
                        for dt in range(2):
                            t = n * 4 + dp * 2 + dt
                            for m in range(N_MCHUNK):
                                nc.tensor.matmul(
                                    pw[:, dt, :N_EMBED],
                                    lhsT=mlpT[:, m, (dp * 2 + dt) * 128:
                                              (dp * 2 + dt + 1) * 128],
                                    rhs=w2all[:, m, :],
                                    start=(m == 0), stop=False)
                            nc.tensor.matmul(pw[:, dt, :N_EMBED], lhsT=ident,
                                             rhs=x[:, t, :],
                                             start=False, stop=True)
                        t0 = n * 4 + dp * 2
                        if has["b2"]:
                            nc.vector.tensor_tensor(
                                out=x[:, t0:t0 + 2, :],
                                in0=pw[:, :, :N_EMBED],
                                in1=bias_sb["b2"][:, None, layer, :]
                                .to_broadcast([128, 2, N_EMBED]), op=OP.add)
                        else:
                            nc.scalar.copy(x[:, t0:t0 + 2, :],
                                           pw[:, :, :N_EMBED])

            # ---- final LN + LM head ----
            xfT = per.tile([128, N_CHUNKS, N_TOK], BF16, tag="ht1")
            layernorm_to(xfT, "lnf")
            wlm_c = []
            for c in range(N_CHUNKS):
                w = wts.tile([128, VOCAB], BF16, tag="wlm", bufs=3)
                nc.sync.dma_start(w, d["wlm"][c * 128:(c + 1) * 128, :])
                wlm_c.append(w)
            for tp in range(N_TILES // 2):
                pl = ps.tile([128, 2, 512], F32, tag="ps")
                for dt in range(2):
                    for c in range(N_CHUNKS):
                        nc.tensor.matmul(
                            pl[:, dt, :VOCAB],
                            lhsT=xfT[:, c, (tp * 2 + dt) * 128:
                                     (tp * 2 + dt + 1) * 128],
                            rhs=wlm_c[c],
                            start=(c == 0), stop=(c == N_CHUNKS - 1))
                lg = wk.tile([128, 2, VOCAB], F32, tag="lg")
                if has["blm"]:
                    nc.vector.tensor_tensor(
                        out=lg, in0=pl[:, :, :VOCAB],
                        in1=bias_sb["blm"][:, None, :].to_broadcast(
                            [128, 2, VOCAB]), op=OP.add)
                else:
                    nc.vector.tensor_copy(lg, pl[:, :, :VOCAB])
                for dt in range(2):
                    t = tp * 2 + dt
                    nc.sync.dma_start(
                        logits_d[t * 128:(t + 1) * 128, :], lg[:, dt, :])

    nc.compile()
    return nc


_CACHE = {}


def _get_nc(has):
    key = tuple(sorted(has.items()))
    if key not in _CACHE:
        _CACHE[key] = _build(has)
    return _CACHE[key]


def kernel(**inputs):
    shared, has, idx_f = _prep(inputs)
    nc = _get_nc(has)
    in_maps = []
    for core in range(N_CORES):
        m = dict(shared)
        m["idxf"] = idx_f[core]
        in_maps.append(m)
    res = run_bass_kernel_spmd(nc, in_maps, core_ids=list(range(N_CORES)))
    out = np.stack([r["logits"].reshape(B_LOC, T, VOCAB) for r in res.results])
    return out.reshape(B, T, VOCAB)


# revision 37
# speedup vs baseline: 1.1440x; 1.0002x over previous
"""Bass/Trainium2 kernel for a 6-layer GPT-style transformer (BigramLanguageModel).

Contract: kernel(**inputs) takes the FULL unsharded inputs from
reference.setup_inputs() and returns the FULL [32, 512, 65] fp32 logits.

Sharding: data-parallel over batch. Each of the 8 NeuronCores runs the whole
model on 4 of the 32 sequences (params replicated); outputs are concatenated
on the host. No collectives.

Device-side design (per core, 2048 tokens), v2 (all-bf16 matmuls):
 - all matmul operands bf16 (weights converted on host; activations written
   bf16 at the PSUM->SBUF copy). PSUM accumulation stays fp32. This enables
   FWL weight loads, 1 cyc/row matmuls everywhere, and 2x/4x DVE modes.
 - residual stream x: fp32 token-major SBUF [128, 16, 384].
 - LayerNorm: bn_stats/bn_aggr (DVE) in groups of 8 token tiles; apply writes
   bf16 h; PE transposes 8 tiles/chunk into one 2-bank PSUM tile; single
   [128,1024] DVE copy to the E-major hT buffer.
 - QKV: QT/KT per head-pair [128, 2048] bf16, head0 on partitions 0:64 and
   head1 on 64:128 (one PSUM->SBUF copy per 2 n-blocks); V token-major into
   v_aug [128, 16, 768] with per-head windows [ones64 | V_h64].
 - attention per (pair, seq): both heads' S^T blocks [128 k, width] computed
   into the two banks of one PSUM tile by row-packed concurrent K=64 matmuls;
   causal mask applied by PRE-ACCUMULATING -1e30 into the diagonal 128 cols
   via an ident@negtri matmul (start=True) so exp(scale*S) lands 0 exactly --
   no DVE mask op, chain is S(PE)->exp(ACT)->AV(PE). Batched exp over both
   heads [128, 2, width]. AV uses the [ones|V] stationary windows so PSUM
   rows 0:64 replicate the softmax denominator -> one batched fast-reciprocal
   + per-head tensor_tensor writes the normalized OT into the E-major concat
   buffer feeding the proj matmul.
 - MLP: mlpT = W1.T @ h2T (E-major), relu fused into the PSUM->SBUF copy
   (bf16), W2 with mlpT chunks stationary, token-major out + residual add.
 - logits: final LN -> xfT -> x @ Wlm per token tile, DMA out [2048, 65].
 - single PSUM pool of [128, 2, 512] (2-bank) tiles, bufs=4 = all 8 banks.
"""

import sys

for _p in ("/opt/trn_rl_repo", "/opt/pypackages"):
    if _p not in sys.path:
        sys.path.insert(0, _p)

import ml_dtypes
import numpy as np

import concourse.bass as bass
import concourse.tile as tile
from concourse import bacc, mybir
from concourse.bass_utils import run_bass_kernel_spmd

F32 = mybir.dt.float32
BF16 = mybir.dt.bfloat16

N_EMBED = 384
CONTEXT = 512
N_HEADS = 6
HEAD_DIM = 64
N_LAYERS = 6
VOCAB = 65
B, T = 32, 512
LN_EPS = 1e-5
N_CORES = 8
B_LOC = B // N_CORES          # 4 sequences per core
N_TOK = B_LOC * T             # 2048 tokens per core
N_TILES = N_TOK // 128        # 16 token tiles
N_CHUNKS = N_EMBED // 128     # 3 E-chunks
N_MLP = 4 * N_EMBED           # 1536
N_MCHUNK = N_MLP // 128       # 12
SCALE = float(N_EMBED) ** -0.5
# Mask addend: scale*NEG ~ -102 -> exp underflows to 0 (exact 0 after bf16
# cast). Huge magnitudes (-1e30) make the HW ACT exp LUT produce NaN.
NEG = -2000.0
DEBUG_L0 = False
V_W = N_HEADS * 128           # 768: per-head [ones64 | V64] windows


def _prep(inputs):
    """Host-side layout prep + exact LN folds. Returns (shared, has, idx)."""
    f = lambda a: np.ascontiguousarray(np.asarray(a), dtype=np.float32)
    bf = lambda a: np.ascontiguousarray(np.asarray(a)).astype(np.float32)
    idx = np.asarray(inputs["idx"])
    tok_emb, pos_emb = f(inputs["tok_emb"]), f(inputs["pos_emb"])
    Wq, Wk, Wv = f(inputs["Wq"]), f(inputs["Wk"]), f(inputs["Wv"])
    Wproj, bproj = f(inputs["Wproj"]), f(inputs["bproj"])
    W1, b1, W2, b2 = f(inputs["W1"]), f(inputs["b1"]), f(inputs["W2"]), f(inputs["b2"])
    ln1_g, ln1_b = f(inputs["ln1_g"]), f(inputs["ln1_b"])
    ln2_g, ln2_b = f(inputs["ln2_g"]), f(inputs["ln2_b"])
    lnf_g, lnf_b = f(inputs["lnf_g"]), f(inputs["lnf_b"])
    Wlm, blm = f(inputs["Wlm"]), f(inputs["blm"])

    L, H, E, D = N_LAYERS, N_HEADS, N_EMBED, HEAD_DIM

    # fold ln gains into the consuming weights (exact when g==1)
    Wq_f = ln1_g[:, None, :, None] * Wq          # [L,H,E,D]
    Wk_f = ln1_g[:, None, :, None] * Wk
    Wv_f = ln1_g[:, None, :, None] * Wv
    W1_f = ln2_g[:, :, None] * W1                # [L,E,4E]
    Wlm_f = lnf_g[:, None] * Wlm                 # [E,V]

    # ln biases propagate through the matmuls as constant bias vectors
    qb = np.einsum("le,lhed->lhd", ln1_b, Wq)    # [L,H,D]
    kb = np.einsum("le,lhed->lhd", ln1_b, Wk)
    vb = np.einsum("le,lhed->lhd", ln1_b, Wv)
    b1_eff = b1 + np.einsum("le,lem->lm", ln2_b, W1)    # [L,4E]
    blm_eff = blm + lnf_b @ Wlm                          # [V]

    # head-pair packed QT/KT weights: [L, 3, E, 128] (pair r = heads 2r, 2r+1)
    wqp = np.concatenate([Wq_f[:, 0::2], Wq_f[:, 1::2]], axis=-1)
    wkp = np.concatenate([Wk_f[:, 0::2], Wk_f[:, 1::2]], axis=-1)
    qbp = np.concatenate([qb[:, 0::2], qb[:, 1::2]], axis=-1)      # [L,3,128]
    kbp = np.concatenate([kb[:, 0::2], kb[:, 1::2]], axis=-1)
    wv_all = Wv_f.transpose(0, 2, 1, 3).reshape(L, E, H * D)       # [L,E,384]
    vb_all = vb.reshape(L, H * D)

    # negtri[k, q] = -1e30 where key k > query q (strict upper kept at 0)
    triu = np.triu(np.ones((128, 128), dtype=np.float32))
    negtri = (1.0 - triu) * NEG

    b16 = lambda a: np.ascontiguousarray(a).astype(ml_dtypes.bfloat16)
    shared = dict(
        tok_emb=b16(tok_emb),
        pos_emb=b16(pos_emb),
        wqp=b16(wqp),
        wkp=b16(wkp),
        wv=b16(wv_all),
        wp=b16(Wproj),
        w1=b16(W1_f),
        w2=b16(W2),
        wlm=b16(Wlm_f),
        ident=b16(np.eye(128, dtype=np.float32)),
        iota=np.arange(VOCAB, dtype=np.float32).reshape(VOCAB, 1),
        negtri=b16(negtri),
    )
    flags = dict(
        qb=qbp if np.any(qbp) else None,
        kb=kbp if np.any(kbp) else None,
        vb=np.broadcast_to(vb_all[:, None, :], (L, 128, H * D)).copy()
        if np.any(vb) else None,
        bp=np.broadcast_to(bproj[:, None, :], (L, 128, E)).copy()
        if np.any(bproj) else None,
        b1=np.ascontiguousarray(b1_eff.reshape(L, N_MCHUNK, 128).transpose(0, 2, 1))
        if np.any(b1_eff) else None,                    # [L,128,12] partition-major
        b2=np.broadcast_to(b2[:, None, :], (L, 128, E)).copy() if np.any(b2) else None,
        blm=np.broadcast_to(blm_eff[None, :], (128, VOCAB)).copy()
        if np.any(blm_eff) else None,
    )
    for k, v in flags.items():
        if v is not None:
            shared[k] = np.ascontiguousarray(v, dtype=np.float32)
    has = {k: (v is not None) for k, v in flags.items()}

    idx_f = idx.astype(np.float32).reshape(N_CORES, N_TOK)
    return shared, has, idx_f


def _build(has):
    nc = bacc.Bacc(trn_type="TRN2", debug=False, num_devices=N_CORES)
    d = {}
    d["idxf"] = nc.dram_tensor("idxf", [N_TOK], F32, kind="ExternalInput")
    d["tok_emb"] = nc.dram_tensor("tok_emb", [VOCAB, N_EMBED], BF16, kind="ExternalInput")
    d["pos_emb"] = nc.dram_tensor("pos_emb", [CONTEXT, N_EMBED], BF16, kind="ExternalInput")
    d["wqp"] = nc.dram_tensor("wqp", [N_LAYERS, 3, N_EMBED, 128], BF16, kind="ExternalInput")
    d["wkp"] = nc.dram_tensor("wkp", [N_LAYERS, 3, N_EMBED, 128], BF16, kind="ExternalInput")
    d["wv"] = nc.dram_tensor("wv", [N_LAYERS, N_EMBED, N_EMBED], BF16, kind="ExternalInput")
    d["wp"] = nc.dram_tensor("wp", [N_LAYERS, N_EMBED, N_EMBED], BF16, kind="ExternalInput")
    d["w1"] = nc.dram_tensor("w1", [N_LAYERS, N_EMBED, N_MLP], BF16, kind="ExternalInput")
    d["w2"] = nc.dram_tensor("w2", [N_LAYERS, N_MLP, N_EMBED], BF16, kind="ExternalInput")
    d["wlm"] = nc.dram_tensor("wlm", [N_EMBED, VOCAB], BF16, kind="ExternalInput")
    d["ident"] = nc.dram_tensor("ident", [128, 128], BF16, kind="ExternalInput")
    d["iota"] = nc.dram_tensor("iota", [VOCAB, 1], F32, kind="ExternalInput")
    d["negtri"] = nc.dram_tensor("negtri", [128, 128], BF16, kind="ExternalInput")
    if has["qb"]:
        d["qb"] = nc.dram_tensor("qb", [N_LAYERS, 3, 128], F32, kind="ExternalInput")
    if has["kb"]:
        d["kb"] = nc.dram_tensor("kb", [N_LAYERS, 3, 128], F32, kind="ExternalInput")
    if has["vb"]:
        d["vb"] = nc.dram_tensor("vb", [N_LAYERS, 128, N_EMBED], F32, kind="ExternalInput")
    if has["bp"]:
        d["bp"] = nc.dram_tensor("bp", [N_LAYERS, 128, N_EMBED], F32, kind="ExternalInput")
    if has["b1"]:
        d["b1"] = nc.dram_tensor("b1", [N_LAYERS, 128, N_MCHUNK], F32, kind="ExternalInput")
    if has["b2"]:
        d["b2"] = nc.dram_tensor("b2", [N_LAYERS, 128, N_EMBED], F32, kind="ExternalInput")
    if has["blm"]:
        d["blm"] = nc.dram_tensor("blm", [128, VOCAB], F32, kind="ExternalInput")
    logits_d = nc.dram_tensor("logits", [N_TOK, VOCAB], F32, kind="ExternalOutput")
    dbg = {}
    if DEBUG_L0:
        dbg["x0"] = nc.dram_tensor("dbg_x0", [128, N_TILES * N_EMBED], BF16, kind="ExternalOutput")
        dbg["ht"] = nc.dram_tensor("dbg_ht", [128, N_CHUNKS * N_TOK], BF16, kind="ExternalOutput")
        dbg["qt"] = nc.dram_tensor("dbg_qt", [128, N_TOK], BF16, kind="ExternalOutput")
        dbg["kt"] = nc.dram_tensor("dbg_kt", [128, N_TOK], BF16, kind="ExternalOutput")
        dbg["va"] = nc.dram_tensor("dbg_va", [128, N_TILES * V_W], BF16, kind="ExternalOutput")
        dbg["at0"] = nc.dram_tensor("dbg_at0", [128, 2 * 512], BF16, kind="ExternalOutput")
        dbg["po0"] = nc.dram_tensor("dbg_po0", [128, 2 * 512], F32, kind="ExternalOutput")
        dbg["rho0"] = nc.dram_tensor("dbg_rho0", [64, 2 * 512], F32, kind="ExternalOutput")
        dbg["otc"] = nc.dram_tensor("dbg_otc", [128, N_CHUNKS * N_TOK], BF16, kind="ExternalOutput")
        dbg["x1"] = nc.dram_tensor("dbg_x1", [128, N_TILES * N_EMBED], BF16, kind="ExternalOutput")

    AF = mybir.ActivationFunctionType
    OP = mybir.AluOpType

    with tile.TileContext(nc) as tc:
        with tc.tile_pool(name="const", bufs=1) as cst, \
             tc.tile_pool(name="persist", bufs=1) as per, \
             tc.tile_pool(name="work", bufs=3) as wk, \
             tc.tile_pool(name="htile", bufs=9) as hp, \
             tc.tile_pool(name="wts", bufs=4) as wts, \
             tc.tile_pool(name="ps", bufs=4, space="PSUM") as ps:

            # ---- constants ----
            ident = cst.tile([128, 128], BF16)
            nc.sync.dma_start(ident, d["ident"][:, :])
            iota = cst.tile([VOCAB, 1], F32)
            nc.sync.dma_start(iota, d["iota"][:, :])
            negtri = cst.tile([128, 128], BF16)
            nc.sync.dma_start(negtri, d["negtri"][:, :])
            eps_sb = cst.tile([128, 1], F32)
            nc.vector.memset(eps_sb, LN_EPS)
            tok_sb = cst.tile([VOCAB, N_EMBED], BF16)
            nc.sync.dma_start(tok_sb, d["tok_emb"][:, :])

            bias_sb = {}
            for nm, shp in (("vb", [128, N_EMBED]), ("bp", [128, N_EMBED]),
                            ("b2", [128, N_EMBED])):
                if has[nm]:
                    bias_sb[nm] = cst.tile([128, N_LAYERS, shp[1]], F32)
                    nc.sync.dma_start(
                        bias_sb[nm],
                        d[nm].rearrange("l p e -> p l e"))
            if has["b1"]:
                bias_sb["b1"] = cst.tile([128, N_LAYERS, N_MCHUNK], F32)
                nc.sync.dma_start(bias_sb["b1"], d["b1"].rearrange("l p m -> p l m"))
            for nm in ("qb", "kb"):
                if has[nm]:
                    bias_sb[nm] = cst.tile([128, N_LAYERS, 3], F32)
                    nc.sync.dma_start(bias_sb[nm], d[nm].rearrange("l r p -> p l r"))
            if has["blm"]:
                bias_sb["blm"] = cst.tile([128, VOCAB], F32)
                nc.sync.dma_start(bias_sb["blm"], d["blm"][:, :])

            # ---- persistent activations ----
            x = per.tile([128, N_TILES, N_EMBED], BF16)         # residual, token-major
            pos_sb = cst.tile([128, B_LOC, N_EMBED], BF16)
            nc.sync.dma_start(
                pos_sb, d["pos_emb"].rearrange("(a p) e -> p a e", p=128))
            v_aug = per.tile([128, N_TILES, V_W], BF16)         # per-head [ones|V]
            ones_blk = cst.tile([128, 64], BF16)
            nc.vector.memset(ones_blk, 1.0)
            for h in range(N_HEADS):                            # ones stripes
                nc.vector.tensor_copy(
                    v_aug[:, :, h * 128:h * 128 + 64],
                    ones_blk[:, None, :].to_broadcast([128, N_TILES, 64]))

            # ---- embedding: x = onehot(idx) @ tok_emb + pos ----
            for tp in range(N_TILES // 2):
                pe = ps.tile([128, 2, 512], F32, tag="ps")
                for dt in range(2):
                    t = tp * 2 + dt
                    idx_b = wk.tile([VOCAB, 128], F32, tag="idxb")
                    nc.sync.dma_start(
                        idx_b,
                        bass.AP(tensor=d["idxf"], offset=t * 128,
                                ap=[[0, VOCAB], [1, 128]]))
                    oh = wk.tile([VOCAB, 128], BF16, tag="oh")
                    nc.vector.tensor_scalar(out=oh, in0=idx_b, scalar1=iota,
                                            scalar2=None, op0=OP.is_equal)
                    nc.tensor.matmul(pe[:, dt, :N_EMBED], lhsT=oh, rhs=tok_sb,
                                     start=True, stop=False)
                    nc.tensor.matmul(pe[:, dt, :N_EMBED], lhsT=ident,
                                     rhs=pos_sb[:, t % B_LOC, :],
                                     start=False, stop=True)
                    nc.scalar.copy(x[:, t, :], pe[:, dt, :N_EMBED])

            LNG = 4

            def ln_group(dst_hT, src_name, tg):
                """One LN group: stats -> rstd -> apply (bf16) -> 12 PE
                transposes into one 2-bank bf16 PSUM tile (chunks 0,1 in
                bank0, chunk 2 in bank1) -> 2 DVE copies to E-major dst."""
                G = LNG
                mvg = wk.tile([128, G, 2], F32, tag="mv" + src_name)
                for dt in range(G):
                    st = wk.tile([128, 6], F32, tag="bnst")
                    nc.vector.bn_stats(out=st, in_=x[:, tg * G + dt, :])
                    nc.vector.bn_aggr(out=mvg[:, dt, :], in_=st)
                sstd = wk.tile([128, G], F32, tag="sstd")
                nc.scalar.activation(out=sstd, in_=mvg[:, :, 1],
                                     func=AF.Sqrt, bias=eps_sb, scale=1.0)
                rstd = wk.tile([128, G], F32, tag="rstd")
                nc.vector.reciprocal(out=rstd, in_=sstd)
                hts = []
                for dt in range(G):
                    t = tg * G + dt
                    ht = hp.tile([128, N_EMBED], BF16, tag="h")
                    nc.vector.tensor_scalar(
                        out=ht, in0=x[:, t, :],
                        scalar1=mvg[:, dt, 0:1], scalar2=rstd[:, dt:dt + 1],
                        op0=OP.subtract, op1=OP.mult)
                    hts.append(ht)
                pt = ps.tile([128, 2, 1024], BF16, tag="ps")
                for c in range(N_CHUNKS):
                    for dt in range(G):
                        nc.tensor.transpose(
                            pt[:, c // 2, (c % 2) * 512 + dt * 128:
                               (c % 2) * 512 + (dt + 1) * 128],
                            hts[dt][:, c * 128:(c + 1) * 128], ident)
                nc.vector.tensor_copy(
                    dst_hT[:, 0:2, tg * G * 128:(tg + 1) * G * 128],
                    pt[:, 0, :].rearrange("p (a b) -> p a b", a=2))
                nc.vector.tensor_copy(
                    dst_hT[:, 2, tg * G * 128:(tg + 1) * G * 128],
                    pt[:, 1, 0:512])

            def layernorm_to(dst_hT, src_name):
                for tg in range(N_TILES // LNG):
                    ln_group(dst_hT, src_name, tg)

            for layer in range(N_LAYERS):
                if DEBUG_L0 and layer == 0:
                    nc.sync.dma_start(dbg["x0"][:, :], x.rearrange("p a b -> p (a b)"))
                hT = per.tile([128, N_CHUNKS, N_TOK], BF16, tag="ht1")
                layernorm_to(hT, "ln1")
                if DEBUG_L0 and layer == 0:
                    nc.sync.dma_start(dbg["ht"][:, :], hT.rearrange("p a b -> p (a b)"))

                # ---- V (token-major, into v_aug [ones|V] windows) ----
                wv_c = []
                for c in range(N_CHUNKS):
                    w = wts.tile([128, N_EMBED], BF16, tag="wchk", bufs=6)
                    nc.sync.dma_start(w, d["wv"][layer, c * 128:(c + 1) * 128, :])
                    wv_c.append(w)
                for tp in range(N_TILES // 2):
                    pv = ps.tile([128, 2, 512], F32, tag="ps")
                    for dt in range(2):
                        for c in range(N_CHUNKS):
                            nc.tensor.matmul(pv[:, dt, :N_EMBED],
                                             lhsT=hT[:, c, (tp * 2 + dt) * 128:
                                                     (tp * 2 + dt + 1) * 128],
                                             rhs=wv_c[c],
                                             start=(c == 0), stop=(c == N_CHUNKS - 1))
                    # scatter [128, 2, 6, 64] -> per-head V slots (offset 64)
                    src = pv[:, :, :N_EMBED].rearrange("p a (h j) -> p a h j", h=6)
                    dst = v_aug[:, tp * 2:tp * 2 + 2, :].rearrange(
                        "p a (h j) -> p a h j", h=6)[:, :, :, 64:128]
                    if has["vb"]:
                        nc.vector.tensor_tensor(
                            out=dst, in0=src,
                            in1=bias_sb["vb"][:, layer, :].rearrange(
                                "p (h j) -> p h j", h=6)[:, None, :, :]
                            .to_broadcast([128, 2, 6, 64]),
                            op=OP.add)
                    else:
                        nc.scalar.copy(dst, src)

                if DEBUG_L0 and layer == 0:
                    nc.sync.dma_start(dbg["va"][:, :], v_aug.rearrange("p a b -> p (a b)"))
                otc = per.tile([128, N_CHUNKS, N_TOK], BF16, tag="otc")

                def emit_qk_chunks(pair):
                    # ---- QT/KT for this pair: [128, 2048] bf16, head0 on
                    # partitions 0:64, head1 on 64:128. Returns (qkt, list of
                    # 4 chunk closures) so chunks can interleave with the
                    # previous pair's attention groups. ----
                    qkt, chunks = {}, []
                    for nm, wd, bias_nm in (("q", d["wqp"], "qb"),
                                            ("k", d["wkp"], "kb")):
                        wqk = wts.tile([128, N_CHUNKS, 128], BF16, tag="wqk",
                                       bufs=4, name=f"wqk_{nm}")
                        for c in range(N_CHUNKS):
                            nc.sync.dma_start(
                                wqk[:, c, :],
                                wd[layer, pair, c * 128:(c + 1) * 128, :])
                        dstT = per.tile([128, N_TOK], BF16, tag=nm + "t",
                                        bufs=2, name=f"qk_{nm}")
                        qkt[nm] = dstT

                        def chunk(np_, wqk=wqk, dstT=dstT, bias_nm=bias_nm):
                            pq = ps.tile([128, 2, 512], F32, tag="ps", name="pq")
                            for half in range(2):
                                n = np_ * 2 + half
                                for c in range(N_CHUNKS):
                                    nc.tensor.matmul(
                                        pq[:, half, :],
                                        lhsT=wqk[:, c, :],
                                        rhs=hT[:, c, n * 512:(n + 1) * 512],
                                        start=(c == 0), stop=(c == N_CHUNKS - 1))
                            for half in range(2):
                                n = np_ * 2 + half
                                dst = dstT[:, n * 512:(n + 1) * 512]
                                if has[bias_nm]:
                                    nc.scalar.activation(
                                        out=dst, in_=pq[:, half, :],
                                        func=AF.Identity,
                                        bias=bias_sb[bias_nm][:, layer,
                                                             pair:pair + 1],
                                        scale=1.0)
                                elif half == 0:
                                    nc.scalar.copy(dst, pq[:, half, :])
                                else:
                                    nc.vector.tensor_copy(dst, pq[:, half, :])

                        for np_ in range(N_TOK // 1024):
                            chunks.append(lambda np_=np_, chunk=chunk: chunk(np_))
                    return qkt, chunks

                def att_group(pair, qkt, s):
                    # ---- attention for one (pair, seq): both heads together.
                    # S^T blocks for head j land in bank j of pa; the causal
                    # mask is pre-accumulated into the diagonal 128 cols. ----
                    if True:
                        po = ps.tile([128, 2, 512], F32, tag="ps", name="po")
                        for ki in range(4):
                            width = 512 - ki * 128
                            kc = s * 512 + ki * 128
                            pa = ps.tile([128, 2, 512], F32, tag="ps", name="pa")
                            at2 = wk.tile([128, 2, 512], BF16, tag="at_sb",
                                          bufs=4, name="at")
                            for j in range(2):
                                nc.tensor.matmul(
                                    pa[:, j, :width],
                                    lhsT=qkt["k"][j * 64:j * 64 + 64, kc:kc + 128],
                                    rhs=qkt["q"][j * 64:j * 64 + 64,
                                                 kc:s * 512 + 512],
                                    start=True, stop=False)
                            # accumulate -1e30 onto the diagonal 128 cols so
                            # exp(scale*S) lands exactly 0 where masked
                            for j in range(2):
                                nc.tensor.matmul(
                                    pa[:, j, 0:128], lhsT=ident, rhs=negtri,
                                    start=False, stop=True)
                            nc.scalar.activation(
                                out=at2[:, :, :width], in_=pa[:, :, :width],
                                func=AF.Exp, scale=SCALE)
                            if DEBUG_L0 and layer == 0 and pair == 0 and s == 0 and ki == 0:
                                nc.sync.dma_start(
                                    dbg["at0"][:, :],
                                    at2.rearrange("p a b -> p (a b)"))
                            for j in range(2):
                                h = 2 * pair + j
                                nc.tensor.matmul(
                                    po[:, j, ki * 128:512],
                                    lhsT=v_aug[:, s * 4 + ki,
                                               h * 128:h * 128 + 128],
                                    rhs=at2[:, j, :width],
                                    start=(ki == 0), stop=(ki == 3))
                        if DEBUG_L0 and layer == 0 and pair == 0 and s == 0:
                            po_sb = wk.tile([128, 2, 512], F32, tag="posb")
                            nc.vector.tensor_copy(po_sb, po)
                            nc.sync.dma_start(
                                dbg["po0"][:, :],
                                po_sb.rearrange("p a b -> p (a b)"))
                        # rows 0:64 of each bank replicate the denominator
                        rho = wk.tile([64, 2, 512], F32, tag="rho", bufs=2,
                                      name="rho")
                        nc.vector.reciprocal_approx_fast(
                            out=rho, in_=po[0:64, :, :])
                        if DEBUG_L0 and layer == 0 and pair == 0 and s == 0:
                            nc.sync.dma_start(
                                dbg["rho0"][:, :],
                                rho.rearrange("p a b -> p (a b)"))
                        for j in range(2):
                            nc.vector.tensor_tensor(
                                out=otc[64 * j:64 * j + 64, pair,
                                        s * 512:(s + 1) * 512],
                                in0=po[64:128, j, :], in1=rho[:, j, :],
                                op=OP.mult)

                # ---- proj weights (prefetch before attention tail) ----
                wp_c = []
                for c in range(N_CHUNKS):
                    w = wts.tile([128, N_EMBED], BF16, tag="wchk", bufs=6)
                    nc.sync.dma_start(w, d["wp"][layer, c * 128:(c + 1) * 128, :])
                    wp_c.append(w)

                def proj_pair(tp):
                    pp = ps.tile([128, 2, 512], F32, tag="ps")
                    for dt in range(2):
                        t = tp * 2 + dt
                        for c in range(N_CHUNKS):
                            nc.tensor.matmul(
                                pp[:, dt, :N_EMBED],
                                lhsT=otc[:, c, t * 128:(t + 1) * 128],
                                rhs=wp_c[c],
                                start=(c == 0), stop=False)
                        # residual: accumulate x into PSUM via identity matmul
                        nc.tensor.matmul(pp[:, dt, :N_EMBED], lhsT=ident,
                                         rhs=x[:, t, :], start=False, stop=True)
                    if has["bp"]:
                        nc.vector.tensor_tensor(
                            out=x[:, tp * 2:tp * 2 + 2, :],
                            in0=pp[:, :, :N_EMBED],
                            in1=bias_sb["bp"][:, None, layer, :]
                            .to_broadcast([128, 2, N_EMBED]), op=OP.add)
                    else:
                        nc.scalar.copy(x[:, tp * 2:tp * 2 + 2, :],
                                       pp[:, :, :N_EMBED])

                # QK of pair p+1 interleaves with attention of pair p (dense
                # PE work hides the exp latency); proj of seq s interleaves
                # with the last pair's attention.
                qkt0, ch0 = emit_qk_chunks(0)
                for ch in ch0:
                    ch()
                qkt1, ch1 = emit_qk_chunks(1)
                for s in range(B_LOC):
                    att_group(0, qkt0, s)
                    ch1[s]()
                qkt2, ch2 = emit_qk_chunks(2)
                for s in range(B_LOC):
                    att_group(1, qkt1, s)
                    ch2[s]()
                for s in range(B_LOC):
                    att_group(2, qkt2, s)
                    proj_pair(2 * s)
                    proj_pair(2 * s + 1)

                if DEBUG_L0 and layer == 0:
                    nc.sync.dma_start(dbg["otc"][:, :], otc.rearrange("p a b -> p (a b)"))
                    for nm_, t_ in (("qt", qkt2["q"]), ("kt", qkt2["k"])):
                        nc.sync.dma_start(dbg[nm_][:, :], t_[:, :])
                    nc.sync.dma_start(dbg["x1"][:, :], x.rearrange("p a b -> p (a b)"))

                # ---- MLP (LN2 group n feeds MLP1/MLP2 of n-block n) ----
                h2T = per.tile([128, N_CHUNKS, N_TOK], BF16, tag="ht2")
                w1all = wts.tile([128, N_CHUNKS, N_MLP], BF16, tag="w1all", bufs=2)
                for c in range(N_CHUNKS):
                    nc.sync.dma_start(
                        w1all[:, c, :], d["w1"][layer, c * 128:(c + 1) * 128, :])
                w2all = wts.tile([128, N_MCHUNK, N_EMBED], BF16, tag="w2all", bufs=2)
                for m in range(N_MCHUNK):
                    nc.sync.dma_start(
                        w2all[:, m, :], d["w2"][layer, m * 128:(m + 1) * 128, :])
                mlpT = per.tile([128, N_MCHUNK, 512], BF16, tag="mlpt")
                for n in range(N_TOK // 512):
                    ln_group(h2T, "ln2", n)
                    for mp in range(N_MCHUNK // 2):
                        pm = ps.tile([128, 2, 512], F32, tag="ps")
                        for dm in range(2):
                            m = mp * 2 + dm
                            for c in range(N_CHUNKS):
                                nc.tensor.matmul(
                                    pm[:, dm, :],
                                    lhsT=w1all[:, c, m * 128:(m + 1) * 128],
                                    rhs=h2T[:, c, n * 512:(n + 1) * 512],
                                    start=(c == 0), stop=(c == N_CHUNKS - 1))
                        if has["b1"]:
                            for dm in range(2):
                                nc.scalar.activation(
                                    out=mlpT[:, mp * 2 + dm, :],
                                    in_=pm[:, dm, :], func=AF.Relu,
                                    bias=bias_sb["b1"][:, layer,
                                                       mp * 2 + dm:mp * 2 + dm + 1],
                                    scale=1.0)
                        else:
                            nc.scalar.activation(
                                out=mlpT[:, mp * 2:mp * 2 + 2, :], in_=pm,
                                func=AF.Relu, scale=1.0)
                    for dp in range(2):
                        pw = ps.tile([128, 2, 512], F32, tag="ps")
                        for dt in range(2):
                            t = n * 4 + dp * 2 + dt
                            for m in range(N_MCHUNK):
                                nc.tensor.matmul(
                                    pw[:, dt, :N_EMBED],
                                    lhsT=mlpT[:, m, (dp * 2 + dt) * 128:
                                              (dp * 2 + dt + 1) * 128],
                                    rhs=w2all[:, m, :],
                                    start=(m == 0), stop=False)
                            nc.tensor.matmul(pw[:, dt, :N_EMBED], lhsT=ident,
                                             rhs=x[:, t, :],
                                             start=False, stop=True)
                        t0 = n * 4 + dp * 2
                        if has["b2"]:
                            nc.vector.tensor_tensor(
                                out=x[:, t0:t0 + 2, :],
                                in0=pw[:, :, :N_EMBED],
                                in1=bias_sb["b2"][:, None, layer, :]
                                .to_broadcast([128, 2, N_EMBED]), op=OP.add)
                        else:
                            nc.scalar.copy(x[:, t0:t0 + 2, :],
                                           pw[:, :, :N_EMBED])

            # ---- final LN + LM head ----
            xfT = per.tile([128, N_CHUNKS, N_TOK], BF16, tag="ht1")
            layernorm_to(xfT, "lnf")
            wlm_c = []
            for c in range(N_CHUNKS):
                w = wts.tile([128, VOCAB], BF16, tag="wlm", bufs=3)
                nc.sync.dma_start(w, d["wlm"][c * 128:(c + 1) * 128, :])
                wlm_c.append(w)
            for tp in range(N_TILES // 2):
                pl = ps.tile([128, 2, 512], F32, tag="ps")
                for dt in range(2):
                    for c in range(N_CHUNKS):
                        nc.tensor.matmul(
                            pl[:, dt, :VOCAB],
                            lhsT=xfT[:, c, (tp * 2 + dt) * 128:
                                     (tp * 2 + dt + 1) * 128],
                            rhs=wlm_c[c],
                            start=(c == 0), stop=(c == N_CHUNKS - 1))
                lg = wk.tile([128, 2, VOCAB], F32, tag="lg")
                if has["blm"]:
                    nc.vector.tensor_tensor(
                        out=lg, in0=pl[:, :, :VOCAB],
                        in1=bias_sb["blm"][:, None, :].to_broadcast(
                            [128, 2, VOCAB]), op=OP.add)
                else:
                    nc.vector.tensor_copy(lg, pl[:, :, :VOCAB])
                for dt in range(2):
                    t = tp * 2 + dt
                    nc.sync.dma_start(
                        logits_d[t * 128:(t + 1) * 128, :], lg[:, dt, :])

    nc.compile()
    return nc


_CACHE = {}


def _get_nc(has):
    key = tuple(sorted(has.items()))
    if key not in _CACHE:
        _CACHE[key] = _build(has)
    return _CACHE[key]


def kernel(**inputs):
    shared, has, idx_f = _prep(inputs)
    nc = _get_nc(has)
    in_maps = []
    for core in range(N_CORES):
        m = dict(shared)
        m["idxf"] = idx_f[core]
        in_maps.append(m)
    res = run_bass_kernel_spmd(nc, in_maps, core_ids=list(range(N_CORES)))
    out = np.stack([r["logits"].reshape(B_LOC, T, VOCAB) for r in res.results])
    return out.reshape(B, T, VOCAB)
